# revision 19
# baseline (speedup 1.0000x reference)
"""Trainium2 Bass kernel for the ButterflyMlp problem.

Computes log_softmax(L3(relu(L2(relu(L1(x)))))) where each Li is a masked
linear layer (butterfly sparsity: global column stripes + a diagonal band),
batch 65536, data-parallel over 8 NeuronCores (8192 rows/core).

Strategy (per core, feature-major throughout):
  - Masks are pre-applied to weights on host. Layer-1 splits into the dense
    stripe GEMM (204 columns shared by all outputs) and a narrow per-block
    band GEMM (<=93 residual columns per 112-row output block).
  - Pass cost on the PE is ~N cycles (N=512 moving columns) regardless of K,
    so the kernel minimizes pass count: fp8e4 DoubleRow contracts 2x128 rows
    per pass. Per 512-column chunk: 7 stripe DR + 7 band plain (L1),
    3 DR + 1 plain (L2: pairs (0,1)(2,3)(4,5) + block 6), 1 fp16 pass (L3),
    1 fp16 pass (exp-sum). Weights are scaled x16 before fp8 quantization;
    the 1/16 folds into eviction scales / w3.
  - HAM clock gate: the PE runs at 1.2 GHz until ~3.4us of *continuous*
    busy time, and any >3.4us idle gap re-throttles it. So the DMA stream
    is ordered to start the first stripe matmul as early as possible
    (per-block ws slivers, then chunk-0 x) and all 16 chunks of x are
    prefetched with 16-deep tile buffers so the PE never starves mid-run.
  - Per chunk, stripe matmuls are emitted in two groups of 4/3 ahead of
    their band matmuls (PSUM limit: 3 pair tiles + 1 single + L2 + L3
    banks = 8) so chunk 0 can start on ws+xs alone before wb/xb land.
  - log_softmax is finished on HOST: the kernel ships z = L3 out (fp16)
    and S = sum(exp(z+b3)) (fp16, via an M=1 ones-matmul into partition 32
    of the same PSUM bank as z, so one [33,512] eviction covers both);
    host computes z + b3 - ln(S). This drops the on-device ln/subtract.
  - Evictions (PSUM fp32 reads are capped at 1 elem/cycle/lane) are
    balanced 5 ops/chunk on ACT (4 relu blocks + exp) and 5 on DVE
    (3 relu blocks + y2 + z/S merged), each ~3.5us/chunk.
  - Bulk DMA via SWDGE (gpsimd ring) in strictly-2D patterns (3D falls
    back to slow GpSimd-ucode copies); small weights + z/S stores ride
    the HWDGE (sync) queue in parallel.
"""
import sys
sys.path.insert(0, "/opt/trn_rl_repo")
import numpy as np
import ml_dtypes

import concourse.bass as bass
import concourse.bacc as bacc
import concourse.mybir as mybir
import concourse.tile as tile
from concourse import bass_utils

F32 = mybir.dt.float32
F16 = mybir.dt.float16
F8 = mybir.dt.float8e4
E4NP = ml_dtypes.float8_e4m3
PM = mybir.MatmulPerfMode.DoubleRow
AF = mybir.ActivationFunctionType
ALU = mybir.AluOpType

# Keep Exp/Relu/Identity/Copy in one ACT table set so the greedy chooser
# emits a single table load instead of reloading twice per chunk.
_PIN_SET = "natural_log_exp_and_others"
_orig_gat = bacc.get_activation_tables


def _pinned_gat(arch):
    tabs = _orig_gat(arch)
    need = {AF.Relu, AF.Identity, AF.Exp, AF.Copy}
    if _PIN_SET in tabs and need <= tabs[_PIN_SET]:
        for name in tabs:
            if name != _PIN_SET:
                tabs[name] = tabs[name] - need
    return tabs


bacc.get_activation_tables = _pinned_gat

N_CORES = 8
NB = 512          # batch columns per matmul (one PSUM bank of fp32)
SC = 512          # batch columns per DMA chunk (= one matmul chunk)
OT = 112          # layer-1 output block width (784/7)
SW = 16.0         # weight pre-scale before fp8 quantization


def _decompose_mask1(mask1):
    """Stripe columns S (true for every row) and per-block residuals R_j."""
    D_out, D_in = mask1.shape
    S = np.where(mask1.all(axis=0))[0]
    n_blk = (D_out + OT - 1) // OT
    stripe_set = np.zeros(D_in, dtype=bool)
    stripe_set[S] = True
    R_list = []
    for j in range(n_blk):
        blk = mask1[j * OT:(j + 1) * OT]
        cols = np.where(blk.any(axis=0) & ~stripe_set)[0]
        assert len(cols) <= 127, f"band block {j} has {len(cols)} cols"
        R_list.append(cols)
    return S, R_list


def _build_program(meta):
    nS, R_lens = meta["nS"], meta["R_lens"]
    Pb = meta["Pb"]                       # band partitions (max R_len + 1)
    Bc = meta["Bc"]
    D1, H, C = meta["D1"], meta["H"], meta["C"]
    n_blk = len(R_lens)
    assert nS % 2 == 0
    hw = nS // 2                          # stripe half width (102)
    n_sup = Bc // SC

    nc = bacc.Bacc("TRN2", target_bir_lowering=False, debug=False,
                   enable_asserts=False, num_devices=N_CORES)

    xs_d = nc.dram_tensor("xs", [hw, n_sup * 2 * SC], F8,
                          kind="ExternalInput").ap()
    xb_d = nc.dram_tensor("xb", [Pb, n_sup * n_blk * SC], F8,
                          kind="ExternalInput").ap()
    ws_d = nc.dram_tensor("ws", [hw, 2 * D1], F8, kind="ExternalInput").ap()
    wb_d = nc.dram_tensor("wb", [Pb, n_blk * OT], F8,
                          kind="ExternalInput").ap()
    w2_d = nc.dram_tensor("w2", [OT, n_blk * H], F8,
                          kind="ExternalInput").ap()
    w3_d = nc.dram_tensor("w3", [H, C], F16, kind="ExternalInput").ap()
    b2_d = nc.dram_tensor("b2", [H, 1], F32, kind="ExternalInput").ap()
    b3_d = nc.dram_tensor("b3", [C, 1], F32, kind="ExternalInput").ap()
    ones_d = nc.dram_tensor("ones", [C, 1], F16, kind="ExternalInput").ap()
    zd_d = nc.dram_tensor("zd", [C, Bc], F16, kind="ExternalOutput").ap()
    sd_d = nc.dram_tensor("sd", [1, Bc], F16, kind="ExternalOutput").ap()

    with tile.TileContext(nc) as tc:
        with tc.tile_pool(name="wp", bufs=1) as wp, \
             tc.tile_pool(name="xp", bufs=16) as xp, \
             tc.tile_pool(name="hp", bufs=2) as hp, \
             tc.tile_pool(name="ep", bufs=2) as ep, \
             tc.tile_pool(name="psp", bufs=2, space="PSUM") as psp, \
             tc.tile_pool(name="ps6", bufs=1, space="PSUM") as ps6, \
             tc.tile_pool(name="ps2", bufs=1, space="PSUM") as ps2, \
             tc.tile_pool(name="psd", bufs=1, space="PSUM") as psd, \
             tc.tile_pool(name="psz", bufs=1, space="PSUM") as psz:

            # ---- HAM warm-up: ~14 dummy matmuls on an uninitialized
            # SBUF tile keep the PE continuously busy from the end of the
            # preamble, so the clock gate reaches 8/8 (~2.4 GHz) before
            # the first real chunk's x lands (~12us). Results land in a
            # dedicated PSUM bank and are never read.
            wdummy = wp.tile([128, NB], F8)
            nc.vector.memset(wdummy[:], 0)
            pd = psd.tile([128, NB], F32, tag="pd", name="pd")
            for _ in range(14):
                nc.tensor.matmul(pd[:], wdummy[:, 0:128], wdummy[:],
                                 start=True, stop=True)

            # ---- small weights ride the HWDGE (sync) queue, in parallel
            # with the SWDGE bulk stream below; z/S stores ride the scalar
            # HWDGE ring so they never queue behind loads.
            w3_sb = wp.tile([H, C], F16)
            nc.sync.dma_start(w3_sb[:], w3_d[:])
            w2_sb = wp.tile([OT, n_blk * H], F8)
            nc.sync.dma_start(w2_sb[:], w2_d[:])
            b2_sb = wp.tile([H, 1], F32)
            nc.sync.dma_start(b2_sb[:], b2_d[:])
            b3_sb = wp.tile([C, 1], F32)
            nc.sync.dma_start(b3_sb[:], b3_d[:])
            ones_sb = wp.tile([C, 1], F16)
            nc.sync.dma_start(ones_sb[:], ones_d[:])
            w2_v = w2_sb[:].rearrange("p (blk h) -> p blk h", blk=n_blk)

            # ---- SWDGE bulk stream (strictly-2D patterns only). ws/wb
            # arrive in HOST-REORDERED block-major layout (block order
            # 6,0,1,..,5 = the kernel's pair order), so chunk-0's first
            # matmul needs only a 23KB ws sliver + xs0, and the rest of
            # chunk 0 streams in pair-sized slivers just ahead of the PE.
            # All x tiles stay resident (bufs=16).
            ws_sb = wp.tile([hw, 2 * D1], F8)
            wsv = ws_sb[:].rearrange("p (blk two m) -> p blk two m",
                                     blk=n_blk, two=2)
            wb_sb = wp.tile([Pb, n_blk * OT], F8)
            xs_tiles, xb_tiles = [], []
            xs_t0 = xp.tile([hw, 2 * SC], F8, name="xs_t", tag="xs")
            xb_t0 = xp.tile([Pb, n_blk * SC], F8, name="xb_t", tag="xb")
            BO = 2 * OT    # flat ws cols per block
            nc.gpsimd.dma_start(ws_sb[:, 0:BO], ws_d[:, 0:BO])       # b6
            nc.gpsimd.dma_start(xs_t0[:], xs_d[:, 0:2 * SC])
            nc.gpsimd.dma_start(wb_sb[:, 0:OT], wb_d[:, 0:OT])       # b6
            nc.gpsimd.dma_start(xb_t0[:, 6 * SC:7 * SC],
                                xb_d[:, 6 * SC:7 * SC])              # b6
            nc.gpsimd.dma_start(ws_sb[:, BO:4 * BO], ws_d[:, BO:4 * BO])
            nc.gpsimd.dma_start(xb_t0[:, 0:2 * SC], xb_d[:, 0:2 * SC])
            nc.gpsimd.dma_start(wb_sb[:, OT:n_blk * OT],
                                wb_d[:, OT:n_blk * OT])
            nc.gpsimd.dma_start(ws_sb[:, 4 * BO:n_blk * BO],
                                ws_d[:, 4 * BO:n_blk * BO])
            nc.gpsimd.dma_start(xb_t0[:, 2 * SC:6 * SC],
                                xb_d[:, 2 * SC:6 * SC])
            xs_tiles.append(xs_t0)
            xb_tiles.append(xb_t0)
            for s in range(1, n_sup):
                xs_t = xp.tile([hw, 2 * SC], F8, name="xs_t", tag="xs")
                nc.gpsimd.dma_start(
                    xs_t[:], xs_d[:, s * 2 * SC:(s + 1) * 2 * SC])
                xb_t = xp.tile([Pb, n_blk * SC], F8, name="xb_t", tag="xb")
                nc.gpsimd.dma_start(
                    xb_t[:], xb_d[:, s * n_blk * SC:(s + 1) * n_blk * SC])
                xs_tiles.append(xs_t)
                xb_tiles.append(xb_t)

            # The L2->L3->softmax tail is software-pipelined across chunks
            # so the PE never waits on an eviction: during chunk s's L1
            # phase the kernel emits lse(s-2)+stores and L2(s-1) after
            # pair01, and L3(s-1)+exp after pair45. Pair 6 runs FIRST so
            # its eviction lands early (it feeds L2's plain pass). Every
            # cross-engine edge gets >=0.7us of slack. z and S share one
            # PSUM bank (z at partitions 0..9, S at 32), so a single
            # [33,NB] DVE eviction covers both; host does z + b3 - ln(S).
            def emit_l2(st):
                y1 = st["y1"]
                p2 = ps2.tile([H, NB], F32, tag="l2", name="p2")
                for kp in range(3):
                    nc.tensor.matmul(p2[:],
                                     w2_v[:, 2 * kp:2 * kp + 2, :],
                                     y1[:, 2 * kp:2 * kp + 2, :],
                                     start=(kp == 0), stop=False,
                                     perf_mode=PM)
                nc.tensor.matmul(p2[:], w2_v[:, 6, :], y1[:, 6, :],
                                 start=False, stop=True)
                # y2 stored at x16 scale (w3 pre-divided by 16 on host);
                # b2 arrives pre-multiplied by 16.
                y2 = hp.tile([H, NB], F16, tag="y2")
                nc.vector.tensor_scalar(y2[:], p2[:], b2_sb[:, 0:1], 0.0,
                                        op0=ALU.add, op1=ALU.max)
                st["y2"] = y2
                return st

            def emit_l3(st):
                pz = psz.tile([33, NB], F32, tag="l3", name="pz")
                nc.tensor.matmul(pz[0:C, :], w3_sb[:], st["y2"][:],
                                 start=True, stop=True)
                ex = ep.tile([C, NB], F16, tag="ex")
                nc.scalar.activation(ex[:], pz[0:C, :], AF.Exp,
                                     bias=b3_sb[:, 0:1])
                st["pz"], st["ex"] = pz, ex
                return st

            def emit_tail(st):
                pz, ex, bs = st["pz"], st["ex"], st["bs"]
                nc.tensor.matmul(pz[32:33, :], ones_sb[:], ex[:],
                                 start=True, stop=True)
                # bufs=16: store receipts lag ~2 chunks; with fewer
                # buffers the DVE eviction stalls on them.
                zs = ep.tile([33, NB], F16, tag="zs", bufs=16)
                nc.vector.tensor_scalar(zs[:], pz[:], 1.0, 0.0,
                                        op0=ALU.mult, op1=ALU.add)
                nc.sync.dma_start(zd_d[:, bs:bs + NB], zs[0:C, :])
                nc.sync.dma_start(sd_d[:, bs:bs + NB], zs[32:33, :])

            PAIRS = ((6,), (0, 1), (2, 3), (4, 5))
            BPOS = {6: 0, 0: 1, 1: 2, 2: 3, 3: 4, 4: 5, 5: 6}
            stage_l2 = stage_l3 = stage_lse = None
            for s in range(n_sup):
                xs_t, xb_t = xs_tiles[s], xb_tiles[s]
                bs = s * SC
                xs_v = xs_t[:].rearrange("p (two c) -> p two c", two=2)

                y1 = hp.tile([OT, n_blk, NB], F8, name="y1", tag="y1")
                for idx, pair in enumerate(PAIRS):
                    if len(pair) == 2:
                        p = psp.tile([OT, 2 * NB], F32, tag="pp", name="pp")
                    else:
                        p = ps6.tile([OT, NB], F32, tag="p6", name="p6")
                    for bi, j in enumerate(pair):
                        nc.tensor.matmul(
                            p[:, bi * NB:(bi + 1) * NB],
                            wsv[:, BPOS[j], :, :],
                            xs_v[:], start=True, stop=False,
                            perf_mode=PM)
                    for bi, j in enumerate(pair):
                        kj = R_lens[j] + 1        # band cols + ones row
                        bj = BPOS[j]
                        nc.tensor.matmul(
                            p[:, bi * NB:(bi + 1) * NB],
                            wb_sb[:kj, bj * OT:(bj + 1) * OT],
                            xb_t[:kj, j * SC:j * SC + NB],
                            start=False, stop=True)
                    for bi, j in enumerate(pair):
                        # evictions: ACT takes blocks 0,2,4 (+exp);
                        # DVE takes 1,3,5,6 (+y2, z/S merge)
                        reg = p[:, bi * NB:(bi + 1) * NB]
                        if j % 2 == 0 and j != 6:
                            nc.scalar.activation(y1[:, j, :], reg, AF.Relu,
                                                 scale=1.0 / SW)
                        else:
                            nc.vector.tensor_scalar(y1[:, j, :], reg,
                                                    1.0 / SW, 0.0,
                                                    op0=ALU.mult,
                                                    op1=ALU.max)
                    if idx == 1:
                        if stage_lse is not None:
                            emit_tail(stage_lse)
                            stage_lse = None
                        if stage_l2 is not None:
                            stage_l3 = emit_l2(stage_l2)
                            stage_l2 = None
                    elif idx == 3:
                        if stage_l3 is not None:
                            stage_lse = emit_l3(stage_l3)
                            stage_l3 = None

                stage_l2 = {"y1": y1, "bs": bs}

            if stage_lse is not None:
                emit_tail(stage_lse)
            emit_tail(emit_l3(emit_l2(stage_l2)))

    nc.compile()
    return nc


_CACHE = {}


def _prepare(x, W1, b1, W2, b2, W3, b3, mask1, mask2, mask3):
    B, D1 = x.shape
    H = W2.shape[0]
    C = W3.shape[0]
    assert B % N_CORES == 0
    Bc = B // N_CORES

    S, R_list = _decompose_mask1(np.asarray(mask1))
    R_lens = [len(r) for r in R_list]
    n_blk = len(R_list)
    Pb = max(R_lens) + 1
    nS = len(S)
    assert nS % 2 == 0
    hw = nS // 2
    n_sup = Bc // SC

    Wm1 = (np.asarray(W1) * np.asarray(mask1)).astype(np.float32)
    Wm2 = (np.asarray(W2) * np.asarray(mask2)).astype(np.float32)
    Wm3 = (np.asarray(W3) * np.asarray(mask3)).astype(np.float32)
    b1 = np.asarray(b1, np.float32)

    # stripe weights, BLOCK-MAJOR [hw, n_blk, 2, OT] fp8, x16, with the
    # kernel's pair order (6,0,1,..,5) so chunk 0 streams in slivers
    blk_order = [6, 0, 1, 2, 3, 4, 5]
    ws = np.zeros((hw, 2, D1), np.float32)
    ws[:, 0, :] = Wm1[:, S[:hw]].T * SW
    ws[:, 1, :] = Wm1[:, S[hw:]].T * SW
    ws_bm = np.zeros((hw, n_blk, 2, OT), np.float32)
    for i, j in enumerate(blk_order):
        ws_bm[:, i] = ws[:, :, j * OT:(j + 1) * OT]
    ws8 = ws_bm.astype(E4NP).reshape(hw, 2 * D1)

    # band weights, same block order [Pb, n_blk*OT] fp8, x16, with b1*16
    # in the ones-row
    wb = np.zeros((Pb, n_blk * OT), np.float32)
    for i, j in enumerate(blk_order):
        R = R_list[j]
        wb[:len(R), i * OT:(i + 1) * OT] = Wm1[j * OT:(j + 1) * OT, R].T * SW
        wb[len(R), i * OT:(i + 1) * OT] = b1[j * OT:(j + 1) * OT] * SW
    wb8 = wb.astype(E4NP)

    # L2 weights [OT, n_blk, H] fp8, x16: DR pairs (0,1)(2,3)(4,5) +
    # plain block 6
    n_kc2 = D1 // OT
    assert n_kc2 == n_blk
    w2t = Wm2.T.reshape(n_kc2, OT, H)     # [7, 112, H]
    w2 = np.zeros((OT, n_blk, H), np.float32)
    for k in range(n_blk):
        w2[:, k, :] = w2t[k] * SW
    w28 = w2.astype(E4NP).reshape(OT, n_blk * H)

    # y2 is stored at x16 scale (the DVE eviction has no spare op for the
    # 1/16), so w3 absorbs the 1/16 and b2 arrives pre-multiplied by 16.
    w316 = np.ascontiguousarray(Wm3.T / SW).astype(np.float16)   # [H, C]
    b2p = (np.asarray(b2, np.float32) * SW).reshape(H, 1)
    b3p = np.asarray(b3, np.float32).reshape(C, 1)

    xT = np.asarray(x, np.float32).T                        # [D1, B]
    # stripe x [hw, 2, B] fp8 -> per-core chunk slabs
    xs_all = np.stack([xT[S[:hw]], xT[S[hw:]]], axis=1).astype(E4NP)
    xs_all = np.ascontiguousarray(
        xs_all.reshape(hw, 2, N_CORES, n_sup, SC).transpose(0, 2, 3, 1, 4))
    # band x [Pb, n_blk, B] fp8 with ones-row at index len(R_j)
    xb_all = np.zeros((Pb, n_blk, B), E4NP)
    for j, R in enumerate(R_list):
        xb_all[:len(R), j] = xT[R].astype(E4NP)
        xb_all[len(R), j] = 1.0
    xb_all = np.ascontiguousarray(
        xb_all.reshape(Pb, n_blk, N_CORES, n_sup, SC).transpose(0, 2, 3, 1, 4))

    meta = dict(nS=nS, R_lens=R_lens, Pb=Pb, Bc=Bc, D1=D1, H=H, C=C,
                b3=np.asarray(b3, np.float32).reshape(C))
    key = (B, D1, H, C, nS, tuple(R_lens))
    if key not in _CACHE:
        _CACHE[key] = _build_program(meta)
    nc = _CACHE[key]

    in_maps = []
    for c in range(N_CORES):
        in_maps.append({
            "xs": xs_all[:, c].reshape(hw, n_sup * 2 * SC),
            "xb": xb_all[:, c].reshape(Pb, n_sup * n_blk * SC),
            "ws": ws8, "wb": wb8, "w2": w28, "w3": w316,
            "b2": b2p, "b3": b3p,
            "ones": np.ones((C, 1), np.float16),
        })
    return nc, in_maps, meta


def _assemble(results, meta):
    zs = [np.asarray(results[c]["zd"], np.float32).T      # [Bc, C]
          for c in range(N_CORES)]
    ss = [np.asarray(results[c]["sd"], np.float32).reshape(-1)
          for c in range(N_CORES)]
    z = np.concatenate(zs, axis=0)
    S = np.concatenate(ss, axis=0)
    out = z + meta["b3"][None, :] - np.log(S)[:, None]
    return out.astype(np.float32)


def kernel(**inputs):
    nc, in_maps, meta = _prepare(**inputs)
    res = bass_utils.run_bass_kernel_spmd(nc, in_maps,
                                          core_ids=list(range(N_CORES)))
    return _assemble(res.results, meta)


def kernel_traced(tmpdir=None, **inputs):
    """Same as kernel() but with NTFF profiling; returns (output, results)."""
    nc, in_maps, meta = _prepare(**inputs)
    res = bass_utils.run_bass_kernel_spmd(nc, in_maps,
                                          core_ids=list(range(N_CORES)),
                                          trace=True, tmpdir=tmpdir)
    return _assemble(res.results, meta), res


# revision 21
# speedup vs baseline: 1.0128x; 1.0128x over previous
"""Trainium2 Bass kernel for the ButterflyMlp problem.

Computes log_softmax(L3(relu(L2(relu(L1(x)))))) where each Li is a masked
linear layer (butterfly sparsity: global column stripes + a diagonal band),
batch 65536, data-parallel over 8 NeuronCores (8192 rows/core).

Strategy (per core, feature-major throughout):
  - Masks are pre-applied to weights on host. Layer-1 splits into the dense
    stripe GEMM (204 columns shared by all outputs) and a narrow per-block
    band GEMM (<=93 residual columns per 112-row output block).
  - Pass cost on the PE is ~N cycles (N=512 moving columns) regardless of K,
    so the kernel minimizes pass count: fp8e4 DoubleRow contracts 2x128 rows
    per pass. Per 512-column chunk: 7 stripe DR + 7 band plain (L1),
    3 DR + 1 plain (L2: pairs (0,1)(2,3)(4,5) + block 6), 1 fp16 pass (L3),
    1 fp16 pass (exp-sum). Weights are scaled x16 before fp8 quantization;
    the 1/16 folds into eviction scales / w3.
  - HAM clock gate: the PE runs at 1.2 GHz until ~3.4us of *continuous*
    busy time, and any >3.4us idle gap re-throttles it. So the DMA stream
    is ordered to start the first stripe matmul as early as possible
    (per-block ws slivers, then chunk-0 x) and all 16 chunks of x are
    prefetched with 16-deep tile buffers so the PE never starves mid-run.
  - Per chunk, stripe matmuls are emitted in two groups of 4/3 ahead of
    their band matmuls (PSUM limit: 3 pair tiles + 1 single + L2 + L3
    banks = 8) so chunk 0 can start on ws+xs alone before wb/xb land.
  - log_softmax is finished on HOST: the kernel ships z = L3 out (fp16)
    and S = sum(exp(z+b3)) (fp16, via an M=1 ones-matmul into partition 32
    of the same PSUM bank as z, so one [33,512] eviction covers both);
    host computes z + b3 - ln(S). This drops the on-device ln/subtract.
  - Evictions (PSUM fp32 reads are capped at 1 elem/cycle/lane) are
    balanced 5 ops/chunk on ACT (4 relu blocks + exp) and 5 on DVE
    (3 relu blocks + y2 + z/S merged), each ~3.5us/chunk.
  - Bulk DMA via SWDGE (gpsimd ring) in strictly-2D patterns (3D falls
    back to slow GpSimd-ucode copies); small weights + z/S stores ride
    the HWDGE (sync) queue in parallel.
"""
import sys
sys.path.insert(0, "/opt/trn_rl_repo")
import numpy as np
import ml_dtypes

import concourse.bass as bass
import concourse.bacc as bacc
import concourse.mybir as mybir
import concourse.tile as tile
from concourse import bass_utils

F32 = mybir.dt.float32
F16 = mybir.dt.float16
F8 = mybir.dt.float8e4
E4NP = ml_dtypes.float8_e4m3
PM = mybir.MatmulPerfMode.DoubleRow
AF = mybir.ActivationFunctionType
ALU = mybir.AluOpType

# Keep Exp/Relu/Identity/Copy in one ACT table set so the greedy chooser
# emits a single table load instead of reloading twice per chunk.
_PIN_SET = "natural_log_exp_and_others"
_orig_gat = bacc.get_activation_tables


def _pinned_gat(arch):
    tabs = _orig_gat(arch)
    need = {AF.Relu, AF.Identity, AF.Exp, AF.Copy}
    if _PIN_SET in tabs and need <= tabs[_PIN_SET]:
        for name in tabs:
            if name != _PIN_SET:
                tabs[name] = tabs[name] - need
    return tabs


bacc.get_activation_tables = _pinned_gat

N_CORES = 8
NB = 512          # batch columns per matmul (one PSUM bank of fp32)
SC = 512          # batch columns per DMA chunk (= one matmul chunk)
OT = 112          # layer-1 output block width (784/7)
SW = 16.0         # weight pre-scale before fp8 quantization


def _decompose_mask1(mask1):
    """Stripe columns S (true for every row) and per-block residuals R_j."""
    D_out, D_in = mask1.shape
    S = np.where(mask1.all(axis=0))[0]
    n_blk = (D_out + OT - 1) // OT
    stripe_set = np.zeros(D_in, dtype=bool)
    stripe_set[S] = True
    R_list = []
    for j in range(n_blk):
        blk = mask1[j * OT:(j + 1) * OT]
        cols = np.where(blk.any(axis=0) & ~stripe_set)[0]
        assert len(cols) <= 127, f"band block {j} has {len(cols)} cols"
        R_list.append(cols)
    return S, R_list


def _build_program(meta):
    nS, R_lens = meta["nS"], meta["R_lens"]
    Pb = meta["Pb"]                       # band partitions (max R_len + 1)
    Bc = meta["Bc"]
    D1, H, C = meta["D1"], meta["H"], meta["C"]
    n_blk = len(R_lens)
    assert nS % 2 == 0
    hw = nS // 2                          # stripe half width (102)
    n_sup = Bc // SC

    nc = bacc.Bacc("TRN2", target_bir_lowering=False, debug=False,
                   enable_asserts=False, num_devices=N_CORES)

    xs_d = nc.dram_tensor("xs", [hw, n_sup * 2 * SC], F8,
                          kind="ExternalInput").ap()
    xb_d = nc.dram_tensor("xb", [Pb, n_sup * n_blk * SC], F8,
                          kind="ExternalInput").ap()
    ws_d = nc.dram_tensor("ws", [hw, 2 * D1], F8, kind="ExternalInput").ap()
    wb_d = nc.dram_tensor("wb", [Pb, n_blk * OT], F8,
                          kind="ExternalInput").ap()
    w2_d = nc.dram_tensor("w2", [OT, n_blk * H], F8,
                          kind="ExternalInput").ap()
    w3_d = nc.dram_tensor("w3", [H, C], F16, kind="ExternalInput").ap()
    b2_d = nc.dram_tensor("b2", [H, 1], F32, kind="ExternalInput").ap()
    b3_d = nc.dram_tensor("b3", [C, 1], F32, kind="ExternalInput").ap()
    ones_d = nc.dram_tensor("ones", [C, 1], F16, kind="ExternalInput").ap()
    zd_d = nc.dram_tensor("zd", [C, Bc], F16, kind="ExternalOutput").ap()
    sd_d = nc.dram_tensor("sd", [1, Bc], F16, kind="ExternalOutput").ap()

    with tile.TileContext(nc) as tc:
        with tc.tile_pool(name="wp", bufs=1) as wp, \
             tc.tile_pool(name="xp", bufs=16) as xp, \
             tc.tile_pool(name="hp", bufs=2) as hp, \
             tc.tile_pool(name="ep", bufs=2) as ep, \
             tc.tile_pool(name="psp", bufs=2, space="PSUM") as psp, \
             tc.tile_pool(name="ps6", bufs=1, space="PSUM") as ps6, \
             tc.tile_pool(name="ps2", bufs=1, space="PSUM") as ps2, \
             tc.tile_pool(name="psd", bufs=1, space="PSUM") as psd, \
             tc.tile_pool(name="psz", bufs=1, space="PSUM") as psz:

            # ---- HAM warm-up: ~14 dummy matmuls on an uninitialized
            # SBUF tile keep the PE continuously busy from the end of the
            # preamble, so the clock gate reaches 8/8 (~2.4 GHz) before
            # the first real chunk's x lands (~12us). Results land in a
            # dedicated PSUM bank and are never read.
            wdummy = wp.tile([128, NB], F8)
            nc.vector.memset(wdummy[:], 0)
            pd = psd.tile([128, NB], F32, tag="pd", name="pd")
            for _ in range(10):
                nc.tensor.matmul(pd[:], wdummy[:, 0:128], wdummy[:],
                                 start=True, stop=True)

            # ---- chunk-0-critical slivers ride the HWDGE (sync) queue —
            # no Q7 descriptor-generation latency, and they stream
            # concurrently with the SWDGE bulk stream below. ws/wb arrive
            # in HOST-REORDERED block-major layout (block order 6,0,..,5
            # = the kernel's pair order) so pair 6 needs only 184KB.
            ws_sb = wp.tile([hw, 2 * D1], F8)
            wsv = ws_sb[:].rearrange("p (blk two m) -> p blk two m",
                                     blk=n_blk, two=2)
            wb_sb = wp.tile([Pb, n_blk * OT], F8)
            xs_tiles, xb_tiles = [], []
            xs_t0 = xp.tile([hw, 2 * SC], F8, name="xs_t", tag="xs")
            xb_t0 = xp.tile([Pb, n_blk * SC], F8, name="xb_t", tag="xb")
            BO = 2 * OT    # flat ws cols per block
            nc.sync.dma_start(ws_sb[:, 0:BO], ws_d[:, 0:BO])         # b6
            nc.sync.dma_start(xs_t0[:], xs_d[:, 0:2 * SC])
            nc.sync.dma_start(wb_sb[:, 0:OT], wb_d[:, 0:OT])         # b6
            nc.sync.dma_start(xb_t0[:, 6 * SC:7 * SC],
                              xb_d[:, 6 * SC:7 * SC])                # b6
            w2_sb = wp.tile([OT, n_blk * H], F8)
            nc.sync.dma_start(w2_sb[:], w2_d[:])
            w3_sb = wp.tile([H, C], F16)
            nc.sync.dma_start(w3_sb[:], w3_d[:])
            b2_sb = wp.tile([H, 1], F32)
            nc.sync.dma_start(b2_sb[:], b2_d[:])
            b3_sb = wp.tile([C, 1], F32)
            nc.sync.dma_start(b3_sb[:], b3_d[:])
            ones_sb = wp.tile([C, 1], F16)
            nc.sync.dma_start(ones_sb[:], ones_d[:])
            w2_v = w2_sb[:].rearrange("p (blk h) -> p blk h", blk=n_blk)

            # ---- SWDGE bulk stream (strictly-2D patterns only); all x
            # tiles stay resident (bufs=16).
            nc.gpsimd.dma_start(ws_sb[:, BO:n_blk * BO],
                                ws_d[:, BO:n_blk * BO])
            nc.gpsimd.dma_start(xb_t0[:, 0:2 * SC], xb_d[:, 0:2 * SC])
            nc.gpsimd.dma_start(wb_sb[:, OT:n_blk * OT],
                                wb_d[:, OT:n_blk * OT])
            nc.gpsimd.dma_start(xb_t0[:, 2 * SC:6 * SC],
                                xb_d[:, 2 * SC:6 * SC])
            xs_tiles.append(xs_t0)
            xb_tiles.append(xb_t0)
            for s in range(1, n_sup):
                xs_t = xp.tile([hw, 2 * SC], F8, name="xs_t", tag="xs")
                nc.gpsimd.dma_start(
                    xs_t[:], xs_d[:, s * 2 * SC:(s + 1) * 2 * SC])
                xb_t = xp.tile([Pb, n_blk * SC], F8, name="xb_t", tag="xb")
                nc.gpsimd.dma_start(
                    xb_t[:], xb_d[:, s * n_blk * SC:(s + 1) * n_blk * SC])
                xs_tiles.append(xs_t)
                xb_tiles.append(xb_t)

            # The L2->L3->softmax tail is software-pipelined across chunks
            # so the PE never waits on an eviction: during chunk s's L1
            # phase the kernel emits lse(s-2)+stores and L2(s-1) after
            # pair01, and L3(s-1)+exp after pair45. Pair 6 runs FIRST so
            # its eviction lands early (it feeds L2's plain pass). Every
            # cross-engine edge gets >=0.7us of slack. z and S share one
            # PSUM bank (z at partitions 0..9, S at 32), so a single
            # [33,NB] DVE eviction covers both; host does z + b3 - ln(S).
            def emit_l2(st):
                y1 = st["y1"]
                p2 = ps2.tile([H, NB], F32, tag="l2", name="p2")
                for kp in range(3):
                    nc.tensor.matmul(p2[:],
                                     w2_v[:, 2 * kp:2 * kp + 2, :],
                                     y1[:, 2 * kp:2 * kp + 2, :],
                                     start=(kp == 0), stop=False,
                                     perf_mode=PM)
                nc.tensor.matmul(p2[:], w2_v[:, 6, :], y1[:, 6, :],
                                 start=False, stop=True)
                # y2 stored at x16 scale (w3 pre-divided by 16 on host);
                # b2 arrives pre-multiplied by 16.
                y2 = hp.tile([H, NB], F16, tag="y2")
                nc.vector.tensor_scalar(y2[:], p2[:], b2_sb[:, 0:1], 0.0,
                                        op0=ALU.add, op1=ALU.max)
                st["y2"] = y2
                return st

            def emit_l3(st):
                pz = psz.tile([33, NB], F32, tag="l3", name="pz")
                nc.tensor.matmul(pz[0:C, :], w3_sb[:], st["y2"][:],
                                 start=True, stop=True)
                ex = ep.tile([C, NB], F16, tag="ex")
                nc.scalar.activation(ex[:], pz[0:C, :], AF.Exp,
                                     bias=b3_sb[:, 0:1])
                st["pz"], st["ex"] = pz, ex
                return st

            def emit_tail(st):
                pz, ex, bs = st["pz"], st["ex"], st["bs"]
                nc.tensor.matmul(pz[32:33, :], ones_sb[:], ex[:],
                                 start=True, stop=True)
                # bufs=16: store receipts lag ~2 chunks; with fewer
                # buffers the DVE eviction stalls on them.
                zs = ep.tile([33, NB], F16, tag="zs", bufs=16)
                nc.vector.tensor_scalar(zs[:], pz[:], 1.0, 0.0,
                                        op0=ALU.mult, op1=ALU.add)
                nc.sync.dma_start(zd_d[:, bs:bs + NB], zs[0:C, :])
                nc.sync.dma_start(sd_d[:, bs:bs + NB], zs[32:33, :])

            PAIRS = ((6,), (0, 1), (2, 3), (4, 5))
            BPOS = {6: 0, 0: 1, 1: 2, 2: 3, 3: 4, 4: 5, 5: 6}
            stage_l2 = stage_l3 = stage_lse = None
            for s in range(n_sup):
                xs_t, xb_t = xs_tiles[s], xb_tiles[s]
                bs = s * SC
                xs_v = xs_t[:].rearrange("p (two c) -> p two c", two=2)

                y1 = hp.tile([OT, n_blk, NB], F8, name="y1", tag="y1")
                for idx, pair in enumerate(PAIRS):
                    if len(pair) == 2:
                        p = psp.tile([OT, 2 * NB], F32, tag="pp", name="pp")
                    else:
                        p = ps6.tile([OT, NB], F32, tag="p6", name="p6")
                    for bi, j in enumerate(pair):
                        nc.tensor.matmul(
                            p[:, bi * NB:(bi + 1) * NB],
                            wsv[:, BPOS[j], :, :],
                            xs_v[:], start=True, stop=False,
                            perf_mode=PM)
                    for bi, j in enumerate(pair):
                        kj = R_lens[j] + 1        # band cols + ones row
                        bj = BPOS[j]
                        nc.tensor.matmul(
                            p[:, bi * NB:(bi + 1) * NB],
                            wb_sb[:kj, bj * OT:(bj + 1) * OT],
                            xb_t[:kj, j * SC:j * SC + NB],
                            start=False, stop=True)
                    for bi, j in enumerate(pair):
                        # evictions: ACT takes blocks 0,2,4 (+exp);
                        # DVE takes 1,3,5,6 (+y2, z/S merge)
                        reg = p[:, bi * NB:(bi + 1) * NB]
                        if j % 2 == 0 and j != 6:
                            nc.scalar.activation(y1[:, j, :], reg, AF.Relu,
                                                 scale=1.0 / SW)
                        else:
                            nc.vector.tensor_scalar(y1[:, j, :], reg,
                                                    1.0 / SW, 0.0,
                                                    op0=ALU.mult,
                                                    op1=ALU.max)
                    if idx == 1:
                        if stage_lse is not None:
                            emit_tail(stage_lse)
                            stage_lse = None
                        if stage_l2 is not None:
                            stage_l3 = emit_l2(stage_l2)
                            stage_l2 = None
                    elif idx == 3:
                        if stage_l3 is not None:
                            stage_lse = emit_l3(stage_l3)
                            stage_l3 = None

                stage_l2 = {"y1": y1, "bs": bs}

            if stage_lse is not None:
                emit_tail(stage_lse)
            emit_tail(emit_l3(emit_l2(stage_l2)))

    nc.compile()
    return nc


_CACHE = {}


def _prepare(x, W1, b1, W2, b2, W3, b3, mask1, mask2, mask3):
    B, D1 = x.shape
    H = W2.shape[0]
    C = W3.shape[0]
    assert B % N_CORES == 0
    Bc = B // N_CORES

    S, R_list = _decompose_mask1(np.asarray(mask1))
    R_lens = [len(r) for r in R_list]
    n_blk = len(R_list)
    Pb = max(R_lens) + 1
    nS = len(S)
    assert nS % 2 == 0
    hw = nS // 2
    n_sup = Bc // SC

    Wm1 = (np.asarray(W1) * np.asarray(mask1)).astype(np.float32)
    Wm2 = (np.asarray(W2) * np.asarray(mask2)).astype(np.float32)
    Wm3 = (np.asarray(W3) * np.asarray(mask3)).astype(np.float32)
    b1 = np.asarray(b1, np.float32)

    # stripe weights, BLOCK-MAJOR [hw, n_blk, 2, OT] fp8, x16, with the
    # kernel's pair order (6,0,1,..,5) so chunk 0 streams in slivers
    blk_order = [6, 0, 1, 2, 3, 4, 5]
    ws = np.zeros((hw, 2, D1), np.float32)
    ws[:, 0, :] = Wm1[:, S[:hw]].T * SW
    ws[:, 1, :] = Wm1[:, S[hw:]].T * SW
    ws_bm = np.zeros((hw, n_blk, 2, OT), np.float32)
    for i, j in enumerate(blk_order):
        ws_bm[:, i] = ws[:, :, j * OT:(j + 1) * OT]
    ws8 = ws_bm.astype(E4NP).reshape(hw, 2 * D1)

    # band weights, same block order [Pb, n_blk*OT] fp8, x16, with b1*16
    # in the ones-row
    wb = np.zeros((Pb, n_blk * OT), np.float32)
    for i, j in enumerate(blk_order):
        R = R_list[j]
        wb[:len(R), i * OT:(i + 1) * OT] = Wm1[j * OT:(j + 1) * OT, R].T * SW
        wb[len(R), i * OT:(i + 1) * OT] = b1[j * OT:(j + 1) * OT] * SW
    wb8 = wb.astype(E4NP)

    # L2 weights [OT, n_blk, H] fp8, x16: DR pairs (0,1)(2,3)(4,5) +
    # plain block 6
    n_kc2 = D1 // OT
    assert n_kc2 == n_blk
    w2t = Wm2.T.reshape(n_kc2, OT, H)     # [7, 112, H]
    w2 = np.zeros((OT, n_blk, H), np.float32)
    for k in range(n_blk):
        w2[:, k, :] = w2t[k] * SW
    w28 = w2.astype(E4NP).reshape(OT, n_blk * H)

    # y2 is stored at x16 scale (the DVE eviction has no spare op for the
    # 1/16), so w3 absorbs the 1/16 and b2 arrives pre-multiplied by 16.
    w316 = np.ascontiguousarray(Wm3.T / SW).astype(np.float16)   # [H, C]
    b2p = (np.asarray(b2, np.float32) * SW).reshape(H, 1)
    b3p = np.asarray(b3, np.float32).reshape(C, 1)

    xT = np.asarray(x, np.float32).T                        # [D1, B]
    # stripe x [hw, 2, B] fp8 -> per-core chunk slabs
    xs_all = np.stack([xT[S[:hw]], xT[S[hw:]]], axis=1).astype(E4NP)
    xs_all = np.ascontiguousarray(
        xs_all.reshape(hw, 2, N_CORES, n_sup, SC).transpose(0, 2, 3, 1, 4))
    # band x [Pb, n_blk, B] fp8 with ones-row at index len(R_j)
    xb_all = np.zeros((Pb, n_blk, B), E4NP)
    for j, R in enumerate(R_list):
        xb_all[:len(R), j] = xT[R].astype(E4NP)
        xb_all[len(R), j] = 1.0
    xb_all = np.ascontiguousarray(
        xb_all.reshape(Pb, n_blk, N_CORES, n_sup, SC).transpose(0, 2, 3, 1, 4))

    meta = dict(nS=nS, R_lens=R_lens, Pb=Pb, Bc=Bc, D1=D1, H=H, C=C,
                b3=np.asarray(b3, np.float32).reshape(C))
    key = (B, D1, H, C, nS, tuple(R_lens))
    if key not in _CACHE:
        _CACHE[key] = _build_program(meta)
    nc = _CACHE[key]

    in_maps = []
    for c in range(N_CORES):
        in_maps.append({
            "xs": xs_all[:, c].reshape(hw, n_sup * 2 * SC),
            "xb": xb_all[:, c].reshape(Pb, n_sup * n_blk * SC),
            "ws": ws8, "wb": wb8, "w2": w28, "w3": w316,
            "b2": b2p, "b3": b3p,
            "ones": np.ones((C, 1), np.float16),
        })
    return nc, in_maps, meta


def _assemble(results, meta):
    zs = [np.asarray(results[c]["zd"], np.float32).T      # [Bc, C]
          for c in range(N_CORES)]
    ss = [np.asarray(results[c]["sd"], np.float32).reshape(-1)
          for c in range(N_CORES)]
    z = np.concatenate(zs, axis=0)
    S = np.concatenate(ss, axis=0)
    out = z + meta["b3"][None, :] - np.log(S)[:, None]
    return out.astype(np.float32)


def kernel(**inputs):
    nc, in_maps, meta = _prepare(**inputs)
    res = bass_utils.run_bass_kernel_spmd(nc, in_maps,
                                          core_ids=list(range(N_CORES)))
    return _assemble(res.results, meta)


def kernel_traced(tmpdir=None, **inputs):
    """Same as kernel() but with NTFF profiling; returns (output, results)."""
    nc, in_maps, meta = _prepare(**inputs)
    res = bass_utils.run_bass_kernel_spmd(nc, in_maps,
                                          core_ids=list(range(N_CORES)),
                                          trace=True, tmpdir=tmpdir)
    return _assemble(res.results, meta), res


# revision 23
# speedup vs baseline: 1.0217x; 1.0088x over previous
"""Trainium2 Bass kernel for the ButterflyMlp problem.

Computes log_softmax(L3(relu(L2(relu(L1(x)))))) where each Li is a masked
linear layer (butterfly sparsity: global column stripes + a diagonal band),
batch 65536, data-parallel over 8 NeuronCores (8192 rows/core).

Strategy (per core, feature-major throughout):
  - Masks are pre-applied to weights on host. Layer-1 splits into the dense
    stripe GEMM (204 columns shared by all outputs) and a narrow per-block
    band GEMM (<=93 residual columns per 112-row output block).
  - Pass cost on the PE is ~N cycles (N=512 moving columns) regardless of K,
    so the kernel minimizes pass count: fp8e4 DoubleRow contracts 2x128 rows
    per pass. Per 512-column chunk: 7 stripe DR + 7 band plain (L1),
    3 DR + 1 plain (L2: pairs (0,1)(2,3)(4,5) + block 6), 1 fp16 pass (L3),
    1 fp16 pass (exp-sum). Weights are scaled x16 before fp8 quantization;
    the 1/16 folds into eviction scales / w3.
  - HAM clock gate: the PE runs at 1.2 GHz until ~3.4us of *continuous*
    busy time, and any >3.4us idle gap re-throttles it. So the DMA stream
    is ordered to start the first stripe matmul as early as possible
    (per-block ws slivers, then chunk-0 x) and all 16 chunks of x are
    prefetched with 16-deep tile buffers so the PE never starves mid-run.
  - Per chunk, stripe matmuls are emitted in two groups of 4/3 ahead of
    their band matmuls (PSUM limit: 3 pair tiles + 1 single + L2 + L3
    banks = 8) so chunk 0 can start on ws+xs alone before wb/xb land.
  - log_softmax is finished on HOST: the kernel ships z = L3 out (fp16)
    and S = sum(exp(z+b3)) (fp16, via an M=1 ones-matmul into partition 32
    of the same PSUM bank as z, so one [33,512] eviction covers both);
    host computes z + b3 - ln(S). This drops the on-device ln/subtract.
  - Evictions (PSUM fp32 reads are capped at 1 elem/cycle/lane) are
    balanced 5 ops/chunk on ACT (4 relu blocks + exp) and 5 on DVE
    (3 relu blocks + y2 + z/S merged), each ~3.5us/chunk.
  - Bulk DMA via SWDGE (gpsimd ring) in strictly-2D patterns (3D falls
    back to slow GpSimd-ucode copies); small weights + z/S stores ride
    the HWDGE (sync) queue in parallel.
"""
import sys
sys.path.insert(0, "/opt/trn_rl_repo")
import numpy as np
import ml_dtypes

import concourse.bass as bass
import concourse.bacc as bacc
import concourse.mybir as mybir
import concourse.tile as tile
from concourse import bass_utils

F32 = mybir.dt.float32
F16 = mybir.dt.float16
F8 = mybir.dt.float8e4
E4NP = ml_dtypes.float8_e4m3
PM = mybir.MatmulPerfMode.DoubleRow
AF = mybir.ActivationFunctionType
ALU = mybir.AluOpType

# Keep Exp/Relu/Identity/Copy in one ACT table set so the greedy chooser
# emits a single table load instead of reloading twice per chunk.
_PIN_SET = "natural_log_exp_and_others"
_orig_gat = bacc.get_activation_tables


def _pinned_gat(arch):
    tabs = _orig_gat(arch)
    need = {AF.Relu, AF.Identity, AF.Exp, AF.Copy}
    if _PIN_SET in tabs and need <= tabs[_PIN_SET]:
        for name in tabs:
            if name != _PIN_SET:
                tabs[name] = tabs[name] - need
    return tabs


bacc.get_activation_tables = _pinned_gat

N_CORES = 8
NB = 512          # batch columns per matmul (one PSUM bank of fp32)
SC = 512          # batch columns per DMA chunk (= one matmul chunk)
OT = 112          # layer-1 output block width (784/7)
SW = 16.0         # weight pre-scale before fp8 quantization


def _decompose_mask1(mask1):
    """Stripe columns S (true for every row) and per-block residuals R_j."""
    D_out, D_in = mask1.shape
    S = np.where(mask1.all(axis=0))[0]
    n_blk = (D_out + OT - 1) // OT
    stripe_set = np.zeros(D_in, dtype=bool)
    stripe_set[S] = True
    R_list = []
    for j in range(n_blk):
        blk = mask1[j * OT:(j + 1) * OT]
        cols = np.where(blk.any(axis=0) & ~stripe_set)[0]
        assert len(cols) <= 127, f"band block {j} has {len(cols)} cols"
        R_list.append(cols)
    return S, R_list


def _build_program(meta):
    nS, R_lens = meta["nS"], meta["R_lens"]
    Pb = meta["Pb"]                       # band partitions (max R_len + 1)
    Bc = meta["Bc"]
    D1, H, C = meta["D1"], meta["H"], meta["C"]
    n_blk = len(R_lens)
    assert nS % 2 == 0
    hw = nS // 2                          # stripe half width (102)
    n_sup = Bc // SC

    nc = bacc.Bacc("TRN2", target_bir_lowering=False, debug=False,
                   enable_asserts=False, num_devices=N_CORES)

    xs_d = nc.dram_tensor("xs", [hw, n_sup * 2 * SC], F8,
                          kind="ExternalInput").ap()
    xb_d = nc.dram_tensor("xb", [Pb, n_sup * n_blk * SC], F8,
                          kind="ExternalInput").ap()
    ws_d = nc.dram_tensor("ws", [hw, 2 * D1], F8, kind="ExternalInput").ap()
    wb_d = nc.dram_tensor("wb", [Pb, n_blk * OT], F8,
                          kind="ExternalInput").ap()
    w2_d = nc.dram_tensor("w2", [OT, n_blk * H], F8,
                          kind="ExternalInput").ap()
    w3_d = nc.dram_tensor("w3", [H, C], F16, kind="ExternalInput").ap()
    b2_d = nc.dram_tensor("b2", [H, 1], F32, kind="ExternalInput").ap()
    b3_d = nc.dram_tensor("b3", [C, 1], F32, kind="ExternalInput").ap()
    ones_d = nc.dram_tensor("ones", [C, 1], F16, kind="ExternalInput").ap()
    zd_d = nc.dram_tensor("zd", [C, Bc], F16, kind="ExternalOutput").ap()
    sd_d = nc.dram_tensor("sd", [1, Bc], F16, kind="ExternalOutput").ap()

    with tile.TileContext(nc) as tc:
        with tc.tile_pool(name="wp", bufs=1) as wp, \
             tc.tile_pool(name="xp", bufs=16) as xp, \
             tc.tile_pool(name="hp", bufs=2) as hp, \
             tc.tile_pool(name="ep", bufs=2) as ep, \
             tc.tile_pool(name="psp", bufs=2, space="PSUM") as psp, \
             tc.tile_pool(name="ps6", bufs=1, space="PSUM") as ps6, \
             tc.tile_pool(name="ps2", bufs=1, space="PSUM") as ps2, \
             tc.tile_pool(name="psd", bufs=1, space="PSUM") as psd, \
             tc.tile_pool(name="psz", bufs=1, space="PSUM") as psz:

            # ---- HAM warm-up: ~14 dummy matmuls on an uninitialized
            # SBUF tile keep the PE continuously busy from the end of the
            # preamble, so the clock gate reaches 8/8 (~2.4 GHz) before
            # the first real chunk's x lands (~12us). Results land in a
            # dedicated PSUM bank and are never read.
            wdummy = wp.tile([128, NB], F8)
            nc.vector.memset(wdummy[:], 0)
            pd = psd.tile([128, NB], F32, tag="pd", name="pd")

            def dummy_mms(n):
                for _ in range(n):
                    nc.tensor.matmul(pd[:], wdummy[:, 0:128], wdummy[:],
                                     start=True, stop=True)

            dummy_mms(10)

            # ---- chunk-0-critical slivers ride the HWDGE (sync) queue —
            # no Q7 descriptor-generation latency, and they stream
            # concurrently with the SWDGE bulk stream below. ws/wb arrive
            # in HOST-REORDERED block-major layout (block order 6,0,..,5
            # = the kernel's pair order) so pair 6 needs only 184KB.
            ws_sb = wp.tile([hw, 2 * D1], F8)
            wsv = ws_sb[:].rearrange("p (blk two m) -> p blk two m",
                                     blk=n_blk, two=2)
            wb_sb = wp.tile([Pb, n_blk * OT], F8)
            xs_tiles, xb_tiles = [], []
            xs_t0 = xp.tile([hw, 2 * SC], F8, name="xs_t", tag="xs")
            xb_t0 = xp.tile([Pb, n_blk * SC], F8, name="xb_t", tag="xb")
            BO = 2 * OT    # flat ws cols per block
            nc.sync.dma_start(ws_sb[:, 0:BO], ws_d[:, 0:BO])         # b6
            nc.sync.dma_start(xs_t0[:], xs_d[:, 0:2 * SC])
            nc.sync.dma_start(wb_sb[:, 0:OT], wb_d[:, 0:OT])         # b6
            nc.sync.dma_start(xb_t0[:, 6 * SC:7 * SC],
                              xb_d[:, 6 * SC:7 * SC])                # b6
            w2_sb = wp.tile([OT, n_blk * H], F8)
            nc.sync.dma_start(w2_sb[:], w2_d[:])
            w3_sb = wp.tile([H, C], F16)
            nc.sync.dma_start(w3_sb[:], w3_d[:])
            b2_sb = wp.tile([H, 1], F32)
            nc.sync.dma_start(b2_sb[:], b2_d[:])
            b3_sb = wp.tile([C, 1], F32)
            nc.sync.dma_start(b3_sb[:], b3_d[:])
            ones_sb = wp.tile([C, 1], F16)
            nc.sync.dma_start(ones_sb[:], ones_d[:])
            w2_v = w2_sb[:].rearrange("p (blk h) -> p blk h", blk=n_blk)

            # ---- SWDGE bulk stream (strictly-2D patterns only); all x
            # tiles stay resident (bufs=16).
            nc.gpsimd.dma_start(ws_sb[:, BO:n_blk * BO],
                                ws_d[:, BO:n_blk * BO])
            nc.gpsimd.dma_start(xb_t0[:, 0:2 * SC], xb_d[:, 0:2 * SC])
            nc.gpsimd.dma_start(wb_sb[:, OT:n_blk * OT],
                                wb_d[:, OT:n_blk * OT])
            nc.gpsimd.dma_start(xb_t0[:, 2 * SC:6 * SC],
                                xb_d[:, 2 * SC:6 * SC])
            xs_tiles.append(xs_t0)
            xb_tiles.append(xb_t0)
            for s in range(1, n_sup):
                xs_t = xp.tile([hw, 2 * SC], F8, name="xs_t", tag="xs")
                nc.gpsimd.dma_start(
                    xs_t[:], xs_d[:, s * 2 * SC:(s + 1) * 2 * SC])
                xb_t = xp.tile([Pb, n_blk * SC], F8, name="xb_t", tag="xb")
                nc.gpsimd.dma_start(
                    xb_t[:], xb_d[:, s * n_blk * SC:(s + 1) * n_blk * SC])
                xs_tiles.append(xs_t)
                xb_tiles.append(xb_t)

            # The L2->L3->softmax tail is software-pipelined across chunks
            # so the PE never waits on an eviction: during chunk s's L1
            # phase the kernel emits lse(s-2)+stores and L2(s-1) after
            # pair01, and L3(s-1)+exp after pair45. Pair 6 runs FIRST so
            # its eviction lands early (it feeds L2's plain pass). Every
            # cross-engine edge gets >=0.7us of slack. z and S share one
            # PSUM bank (z at partitions 0..9, S at 32), so a single
            # [33,NB] DVE eviction covers both; host does z + b3 - ln(S).
            def emit_l2(st):
                y1 = st["y1"]
                p2 = ps2.tile([H, NB], F32, tag="l2", name="p2")
                for kp in range(3):
                    nc.tensor.matmul(p2[:],
                                     w2_v[:, 2 * kp:2 * kp + 2, :],
                                     y1[:, 2 * kp:2 * kp + 2, :],
                                     start=(kp == 0), stop=False,
                                     perf_mode=PM)
                nc.tensor.matmul(p2[:], w2_v[:, 6, :], y1[:, 6, :],
                                 start=False, stop=True)
                # y2 stored at x16 scale (w3 pre-divided by 16 on host);
                # b2 arrives pre-multiplied by 16.
                y2 = hp.tile([H, NB], F16, tag="y2")
                nc.vector.tensor_scalar(y2[:], p2[:], b2_sb[:, 0:1], 0.0,
                                        op0=ALU.add, op1=ALU.max)
                st["y2"] = y2
                return st

            def emit_l3(st):
                pz = psz.tile([33, NB], F32, tag="l3", name="pz")
                nc.tensor.matmul(pz[0:C, :], w3_sb[:], st["y2"][:],
                                 start=True, stop=True)
                ex = ep.tile([C, NB], F16, tag="ex")
                nc.scalar.activation(ex[:], pz[0:C, :], AF.Exp,
                                     bias=b3_sb[:, 0:1])
                st["pz"], st["ex"] = pz, ex
                return st

            def emit_tail(st):
                pz, ex, bs = st["pz"], st["ex"], st["bs"]
                nc.tensor.matmul(pz[32:33, :], ones_sb[:], ex[:],
                                 start=True, stop=True)
                # bufs=16: store receipts lag ~2 chunks; with fewer
                # buffers the DVE eviction stalls on them.
                zs = ep.tile([33, NB], F16, tag="zs", bufs=16)
                nc.vector.tensor_scalar(zs[:], pz[:], 1.0, 0.0,
                                        op0=ALU.mult, op1=ALU.add)
                nc.sync.dma_start(zd_d[:, bs:bs + NB], zs[0:C, :])
                nc.sync.dma_start(sd_d[:, bs:bs + NB], zs[32:33, :])

            PAIRS = ((6,), (0, 1), (2, 3), (4, 5))
            BPOS = {6: 0, 0: 1, 1: 2, 2: 3, 3: 4, 4: 5, 5: 6}
            stage_l2 = stage_l3 = stage_lse = None
            for s in range(n_sup):
                xs_t, xb_t = xs_tiles[s], xb_tiles[s]
                bs = s * SC
                xs_v = xs_t[:].rearrange("p (two c) -> p two c", two=2)

                y1 = hp.tile([OT, n_blk, NB], F8, name="y1", tag="y1")
                for idx, pair in enumerate(PAIRS):
                    if len(pair) == 2:
                        p = psp.tile([OT, 2 * NB], F32, tag="pp", name="pp")
                    else:
                        p = ps6.tile([OT, NB], F32, tag="p6", name="p6")
                    for bi, j in enumerate(pair):
                        nc.tensor.matmul(
                            p[:, bi * NB:(bi + 1) * NB],
                            wsv[:, BPOS[j], :, :],
                            xs_v[:], start=True, stop=False,
                            perf_mode=PM)
                    for bi, j in enumerate(pair):
                        kj = R_lens[j] + 1        # band cols + ones row
                        bj = BPOS[j]
                        nc.tensor.matmul(
                            p[:, bi * NB:(bi + 1) * NB],
                            wb_sb[:kj, bj * OT:(bj + 1) * OT],
                            xb_t[:kj, j * SC:j * SC + NB],
                            start=False, stop=True)
                    for bi, j in enumerate(pair):
                        # evictions: ACT takes blocks 0,2,4 (+exp);
                        # DVE takes 1,3,5,6 (+y2, z/S merge)
                        reg = p[:, bi * NB:(bi + 1) * NB]
                        if j % 2 == 0 and j != 6:
                            nc.scalar.activation(y1[:, j, :], reg, AF.Relu,
                                                 scale=1.0 / SW)
                        else:
                            nc.vector.tensor_scalar(y1[:, j, :], reg,
                                                    1.0 / SW, 0.0,
                                                    op0=ALU.mult,
                                                    op1=ALU.max)
                    if idx == 1:
                        if stage_lse is not None:
                            emit_tail(stage_lse)
                            stage_lse = None
                        if stage_l2 is not None:
                            stage_l3 = emit_l2(stage_l2)
                            stage_l2 = None
                    elif idx == 3:
                        if stage_l3 is not None:
                            stage_lse = emit_l3(stage_l3)
                            stage_l3 = None
                    if s == 0:
                        # chunk 0 is paced by the DMA ramp: fill the
                        # known feed-lag windows so the HAM clock gate
                        # never sees an idle window and re-throttles.
                        dummy_mms(3)

                stage_l2 = {"y1": y1, "bs": bs}
                if s == 0:
                    dummy_mms(16)
                elif s == 1:
                    dummy_mms(4)

            if stage_lse is not None:
                emit_tail(stage_lse)
            emit_tail(emit_l3(emit_l2(stage_l2)))

    nc.compile()
    return nc


_CACHE = {}


def _prepare(x, W1, b1, W2, b2, W3, b3, mask1, mask2, mask3):
    B, D1 = x.shape
    H = W2.shape[0]
    C = W3.shape[0]
    assert B % N_CORES == 0
    Bc = B // N_CORES

    S, R_list = _decompose_mask1(np.asarray(mask1))
    R_lens = [len(r) for r in R_list]
    n_blk = len(R_list)
    Pb = max(R_lens) + 1
    nS = len(S)
    assert nS % 2 == 0
    hw = nS // 2
    n_sup = Bc // SC

    Wm1 = (np.asarray(W1) * np.asarray(mask1)).astype(np.float32)
    Wm2 = (np.asarray(W2) * np.asarray(mask2)).astype(np.float32)
    Wm3 = (np.asarray(W3) * np.asarray(mask3)).astype(np.float32)
    b1 = np.asarray(b1, np.float32)

    # stripe weights, BLOCK-MAJOR [hw, n_blk, 2, OT] fp8, x16, with the
    # kernel's pair order (6,0,1,..,5) so chunk 0 streams in slivers
    blk_order = [6, 0, 1, 2, 3, 4, 5]
    ws = np.zeros((hw, 2, D1), np.float32)
    ws[:, 0, :] = Wm1[:, S[:hw]].T * SW
    ws[:, 1, :] = Wm1[:, S[hw:]].T * SW
    ws_bm = np.zeros((hw, n_blk, 2, OT), np.float32)
    for i, j in enumerate(blk_order):
        ws_bm[:, i] = ws[:, :, j * OT:(j + 1) * OT]
    ws8 = ws_bm.astype(E4NP).reshape(hw, 2 * D1)

    # band weights, same block order [Pb, n_blk*OT] fp8, x16, with b1*16
    # in the ones-row
    wb = np.zeros((Pb, n_blk * OT), np.float32)
    for i, j in enumerate(blk_order):
        R = R_list[j]
        wb[:len(R), i * OT:(i + 1) * OT] = Wm1[j * OT:(j + 1) * OT, R].T * SW
        wb[len(R), i * OT:(i + 1) * OT] = b1[j * OT:(j + 1) * OT] * SW
    wb8 = wb.astype(E4NP)

    # L2 weights [OT, n_blk, H] fp8, x16: DR pairs (0,1)(2,3)(4,5) +
    # plain block 6
    n_kc2 = D1 // OT
    assert n_kc2 == n_blk
    w2t = Wm2.T.reshape(n_kc2, OT, H)     # [7, 112, H]
    w2 = np.zeros((OT, n_blk, H), np.float32)
    for k in range(n_blk):
        w2[:, k, :] = w2t[k] * SW
    w28 = w2.astype(E4NP).reshape(OT, n_blk * H)

    # y2 is stored at x16 scale (the DVE eviction has no spare op for the
    # 1/16), so w3 absorbs the 1/16 and b2 arrives pre-multiplied by 16.
    w316 = np.ascontiguousarray(Wm3.T / SW).astype(np.float16)   # [H, C]
    b2p = (np.asarray(b2, np.float32) * SW).reshape(H, 1)
    b3p = np.asarray(b3, np.float32).reshape(C, 1)

    xT = np.asarray(x, np.float32).T                        # [D1, B]
    # stripe x [hw, 2, B] fp8 -> per-core chunk slabs
    xs_all = np.stack([xT[S[:hw]], xT[S[hw:]]], axis=1).astype(E4NP)
    xs_all = np.ascontiguousarray(
        xs_all.reshape(hw, 2, N_CORES, n_sup, SC).transpose(0, 2, 3, 1, 4))
    # band x [Pb, n_blk, B] fp8 with ones-row at index len(R_j)
    xb_all = np.zeros((Pb, n_blk, B), E4NP)
    for j, R in enumerate(R_list):
        xb_all[:len(R), j] = xT[R].astype(E4NP)
        xb_all[len(R), j] = 1.0
    xb_all = np.ascontiguousarray(
        xb_all.reshape(Pb, n_blk, N_CORES, n_sup, SC).transpose(0, 2, 3, 1, 4))

    meta = dict(nS=nS, R_lens=R_lens, Pb=Pb, Bc=Bc, D1=D1, H=H, C=C,
                b3=np.asarray(b3, np.float32).reshape(C))
    key = (B, D1, H, C, nS, tuple(R_lens))
    if key not in _CACHE:
        _CACHE[key] = _build_program(meta)
    nc = _CACHE[key]

    in_maps = []
    for c in range(N_CORES):
        in_maps.append({
            "xs": xs_all[:, c].reshape(hw, n_sup * 2 * SC),
            "xb": xb_all[:, c].reshape(Pb, n_sup * n_blk * SC),
            "ws": ws8, "wb": wb8, "w2": w28, "w3": w316,
            "b2": b2p, "b3": b3p,
            "ones": np.ones((C, 1), np.float16),
        })
    return nc, in_maps, meta


def _assemble(results, meta):
    zs = [np.asarray(results[c]["zd"], np.float32).T      # [Bc, C]
          for c in range(N_CORES)]
    ss = [np.asarray(results[c]["sd"], np.float32).reshape(-1)
          for c in range(N_CORES)]
    z = np.concatenate(zs, axis=0)
    S = np.concatenate(ss, axis=0)
    out = z + meta["b3"][None, :] - np.log(S)[:, None]
    return out.astype(np.float32)


def kernel(**inputs):
    nc, in_maps, meta = _prepare(**inputs)
    res = bass_utils.run_bass_kernel_spmd(nc, in_maps,
                                          core_ids=list(range(N_CORES)))
    return _assemble(res.results, meta)


def kernel_traced(tmpdir=None, **inputs):
    """Same as kernel() but with NTFF profiling; returns (output, results)."""
    nc, in_maps, meta = _prepare(**inputs)
    res = bass_utils.run_bass_kernel_spmd(nc, in_maps,
                                          core_ids=list(range(N_CORES)),
                                          trace=True, tmpdir=tmpdir)
    return _assemble(res.results, meta), res


# revision 31
# speedup vs baseline: 1.0403x; 1.0183x over previous
"""Trainium2 Bass kernel for the ButterflyMlp problem.

Computes log_softmax(L3(relu(L2(relu(L1(x)))))) where each Li is a masked
linear layer (butterfly sparsity: global column stripes + a diagonal band),
batch 65536, data-parallel over 8 NeuronCores (8192 rows/core).

Strategy (per core, feature-major throughout):
  - Masks are pre-applied to weights on host. Layer-1 splits into the dense
    stripe GEMM (204 columns shared by all outputs) and a narrow per-block
    band GEMM (<=93 residual columns per 112-row output block).
  - Pass cost on the PE is ~N cycles (N=512 moving columns) regardless of K,
    so the kernel minimizes pass count: fp8e4 DoubleRow contracts 2x128 rows
    per pass. Per 512-column chunk: 7 stripe DR + 7 band plain (L1),
    3 DR + 1 plain (L2: pairs (0,1)(2,3)(4,5) + block 6), 1 fp16 pass (L3),
    1 fp16 pass (exp-sum). Weights are scaled x16 before fp8 quantization;
    the 1/16 folds into eviction scales / w3.
  - HAM clock gate: the PE runs at 1.2 GHz until ~3.4us of *continuous*
    busy time, and any >3.4us idle gap re-throttles it. So the DMA stream
    is ordered to start the first stripe matmul as early as possible
    (per-block ws slivers, then chunk-0 x) and all 16 chunks of x are
    prefetched with 16-deep tile buffers so the PE never starves mid-run.
  - Per chunk, stripe matmuls are emitted in two groups of 4/3 ahead of
    their band matmuls (PSUM limit: 3 pair tiles + 1 single + L2 + L3
    banks = 8) so chunk 0 can start on ws+xs alone before wb/xb land.
  - log_softmax is finished on HOST: the kernel ships z = L3 out (fp16)
    and S = sum(exp(z+b3)) (fp16, via an M=1 ones-matmul into partition 32
    of the same PSUM bank as z, so one [33,512] eviction covers both);
    host computes z + b3 - ln(S). This drops the on-device ln/subtract.
  - Evictions (PSUM fp32 reads are capped at 1 elem/cycle/lane) are
    balanced 5 ops/chunk on ACT (4 relu blocks + exp) and 5 on DVE
    (3 relu blocks + y2 + z/S merged), each ~3.5us/chunk.
  - Bulk DMA via SWDGE (gpsimd ring) in strictly-2D patterns (3D falls
    back to slow GpSimd-ucode copies); small weights + z/S stores ride
    the HWDGE (sync) queue in parallel.
"""
import sys
sys.path.insert(0, "/opt/trn_rl_repo")
import numpy as np
import ml_dtypes

import concourse.bass as bass
import concourse.bacc as bacc
import concourse.mybir as mybir
import concourse.tile as tile
from concourse import bass_utils

F32 = mybir.dt.float32
F16 = mybir.dt.float16
F8 = mybir.dt.float8e4
E4NP = ml_dtypes.float8_e4m3
PM = mybir.MatmulPerfMode.DoubleRow
AF = mybir.ActivationFunctionType
ALU = mybir.AluOpType

# Keep Exp/Relu/Identity/Copy in one ACT table set so the greedy chooser
# emits a single table load instead of reloading twice per chunk.
_PIN_SET = "natural_log_exp_and_others"
_orig_gat = bacc.get_activation_tables


def _pinned_gat(arch):
    tabs = _orig_gat(arch)
    need = {AF.Relu, AF.Identity, AF.Exp, AF.Copy}
    if _PIN_SET in tabs and need <= tabs[_PIN_SET]:
        for name in tabs:
            if name != _PIN_SET:
                tabs[name] = tabs[name] - need
    return tabs


bacc.get_activation_tables = _pinned_gat

N_CORES = 8
NB = 512          # batch columns per matmul (one PSUM bank of fp32)
SC = 512          # batch columns per DMA chunk (= one matmul chunk)
OT = 112          # layer-1 output block width (784/7)
SW = 16.0         # weight pre-scale before fp8 quantization


def _decompose_mask1(mask1):
    """Stripe columns S (true for every row) and per-block residuals R_j."""
    D_out, D_in = mask1.shape
    S = np.where(mask1.all(axis=0))[0]
    n_blk = (D_out + OT - 1) // OT
    stripe_set = np.zeros(D_in, dtype=bool)
    stripe_set[S] = True
    R_list = []
    for j in range(n_blk):
        blk = mask1[j * OT:(j + 1) * OT]
        cols = np.where(blk.any(axis=0) & ~stripe_set)[0]
        assert len(cols) <= 127, f"band block {j} has {len(cols)} cols"
        R_list.append(cols)
    return S, R_list


def _build_program(meta):
    nS, R_lens = meta["nS"], meta["R_lens"]
    Pb = meta["Pb"]                       # band partitions (max R_len + 1)
    Bc = meta["Bc"]
    D1, H, C = meta["D1"], meta["H"], meta["C"]
    n_blk = len(R_lens)
    assert nS % 2 == 0
    hw = nS // 2                          # stripe half width (102)
    n_sup = Bc // SC

    nc = bacc.Bacc("TRN2", target_bir_lowering=False, debug=False,
                   enable_asserts=False, num_devices=N_CORES)

    xs_d = nc.dram_tensor("xs", [hw, n_sup * 2 * SC], F8,
                          kind="ExternalInput").ap()
    xb_d = nc.dram_tensor("xb", [Pb, n_sup * n_blk * SC], F8,
                          kind="ExternalInput").ap()
    ws_d = nc.dram_tensor("ws", [hw, 2 * D1], F8, kind="ExternalInput").ap()
    wb_d = nc.dram_tensor("wb", [Pb, n_blk * OT], F8,
                          kind="ExternalInput").ap()
    w2_d = nc.dram_tensor("w2", [OT, n_blk * H], F8,
                          kind="ExternalInput").ap()
    w3_d = nc.dram_tensor("w3", [H, C], F16, kind="ExternalInput").ap()
    b2_d = nc.dram_tensor("b2", [H, 1], F32, kind="ExternalInput").ap()
    b3_d = nc.dram_tensor("b3", [C, 1], F32, kind="ExternalInput").ap()
    ones_d = nc.dram_tensor("ones", [C, 1], F16, kind="ExternalInput").ap()
    zd_d = nc.dram_tensor("zd", [C, Bc], F16, kind="ExternalOutput").ap()
    sd_d = nc.dram_tensor("sd", [1, Bc], F16, kind="ExternalOutput").ap()

    with tile.TileContext(nc) as tc:
        with tc.tile_pool(name="wp", bufs=1) as wp, \
             tc.tile_pool(name="xp", bufs=16) as xp, \
             tc.tile_pool(name="hp", bufs=2) as hp, \
             tc.tile_pool(name="ep", bufs=2) as ep, \
             tc.tile_pool(name="psp", bufs=2, space="PSUM") as psp, \
             tc.tile_pool(name="ps6", bufs=1, space="PSUM") as ps6, \
             tc.tile_pool(name="ps2", bufs=1, space="PSUM") as ps2, \
             tc.tile_pool(name="psd", bufs=1, space="PSUM") as psd, \
             tc.tile_pool(name="psz", bufs=1, space="PSUM") as psz:

            # ---- HAM warm-up: ~14 dummy matmuls on an uninitialized
            # SBUF tile keep the PE continuously busy from the end of the
            # preamble, so the clock gate reaches 8/8 (~2.4 GHz) before
            # the first real chunk's x lands (~12us). Results land in a
            # dedicated PSUM bank and are never read.
            wdummy = wp.tile([128, NB], F8)
            nc.vector.memset(wdummy[:], 0)
            pd = psd.tile([128, NB], F32, tag="pd", name="pd")

            def dummy_mms(n):
                for _ in range(n):
                    nc.tensor.matmul(pd[:], wdummy[:, 0:128], wdummy[:],
                                     start=True, stop=True)

            dummy_mms(10)

            # ---- chunk-0-critical slivers ride the HWDGE (sync) queue —
            # no Q7 descriptor-generation latency, and they stream
            # concurrently with the SWDGE bulk stream below. ws/wb arrive
            # in HOST-REORDERED block-major layout (block order 6,0,..,5
            # = the kernel's pair order) so pair 6 needs only 184KB.
            ws_sb = wp.tile([hw, 2 * D1], F8)
            wsv = ws_sb[:].rearrange("p (blk two m) -> p blk two m",
                                     blk=n_blk, two=2)
            wb_sb = wp.tile([Pb, n_blk * OT], F8)
            xs_tiles, xb_tiles = [], []
            xs_t0 = xp.tile([hw, 2 * SC], F8, name="xs_t", tag="xs")
            xb_t0 = xp.tile([Pb, n_blk * SC], F8, name="xb_t", tag="xb")
            BO = 2 * OT    # flat ws cols per block
            nc.sync.dma_start(ws_sb[:, 0:BO], ws_d[:, 0:BO])         # b6
            nc.sync.dma_start(xs_t0[:], xs_d[:, 0:2 * SC])
            nc.sync.dma_start(wb_sb[:, 0:OT], wb_d[:, 0:OT])         # b6
            nc.sync.dma_start(xb_t0[:, 6 * SC:7 * SC],
                              xb_d[:, 6 * SC:7 * SC])                # b6
            w2_sb = wp.tile([OT, n_blk * H], F8)
            nc.sync.dma_start(w2_sb[:], w2_d[:])
            w3_sb = wp.tile([H, C], F16)
            nc.sync.dma_start(w3_sb[:], w3_d[:])
            b2_sb = wp.tile([H, 1], F32)
            nc.sync.dma_start(b2_sb[:], b2_d[:])
            b3_sb = wp.tile([C, 1], F32)
            nc.sync.dma_start(b3_sb[:], b3_d[:])
            ones_sb = wp.tile([C, 1], F16)
            nc.sync.dma_start(ones_sb[:], ones_d[:])
            w2_v = w2_sb[:].rearrange("p (blk h) -> p blk h", blk=n_blk)

            # ---- SWDGE bulk stream (strictly-2D patterns only); all x
            # tiles stay resident (bufs=16).
            nc.gpsimd.dma_start(ws_sb[:, BO:n_blk * BO],
                                ws_d[:, BO:n_blk * BO])
            nc.gpsimd.dma_start(xb_t0[:, 0:2 * SC], xb_d[:, 0:2 * SC])
            nc.gpsimd.dma_start(wb_sb[:, OT:n_blk * OT],
                                wb_d[:, OT:n_blk * OT])
            nc.gpsimd.dma_start(xb_t0[:, 2 * SC:6 * SC],
                                xb_d[:, 2 * SC:6 * SC])
            xs_tiles.append(xs_t0)
            xb_tiles.append(xb_t0)
            for s in range(1, n_sup):
                xs_t = xp.tile([hw, 2 * SC], F8, name="xs_t", tag="xs")
                nc.gpsimd.dma_start(
                    xs_t[:], xs_d[:, s * 2 * SC:(s + 1) * 2 * SC])
                xb_t = xp.tile([Pb, n_blk * SC], F8, name="xb_t", tag="xb")
                nc.gpsimd.dma_start(
                    xb_t[:], xb_d[:, s * n_blk * SC:(s + 1) * n_blk * SC])
                xs_tiles.append(xs_t)
                xb_tiles.append(xb_t)

            # The L2->L3->softmax tail is software-pipelined across chunks
            # so the PE never waits on an eviction: during chunk s's L1
            # phase the kernel emits lse(s-2)+stores and L2(s-1) after
            # pair01, and L3(s-1)+exp after pair45. Pair 6 runs FIRST so
            # its eviction lands early (it feeds L2's plain pass). Every
            # cross-engine edge gets >=0.7us of slack. z and S share one
            # PSUM bank (z at partitions 0..9, S at 32), so a single
            # [33,NB] DVE eviction covers both; host does z + b3 - ln(S).
            def emit_l2(st):
                y1 = st["y1"]
                p2 = ps2.tile([H, NB], F32, tag="l2", name="p2")
                for kp in range(3):
                    nc.tensor.matmul(p2[:],
                                     w2_v[:, 2 * kp:2 * kp + 2, :],
                                     y1[:, 2 * kp:2 * kp + 2, :],
                                     start=(kp == 0), stop=False,
                                     perf_mode=PM)
                nc.tensor.matmul(p2[:], w2_v[:, 6, :], y1[:, 6, :],
                                 start=False, stop=True)
                # y2 stored at x16 scale (w3 pre-divided by 16 on host);
                # b2 arrives pre-multiplied by 16.
                y2 = hp.tile([H, NB], F16, tag="y2")
                nc.vector.tensor_scalar(y2[:], p2[:], b2_sb[:, 0:1], 0.0,
                                        op0=ALU.add, op1=ALU.max)
                st["y2"] = y2
                return st

            def emit_l3(st):
                pz = psz.tile([33, NB], F32, tag="l3", name="pz")
                nc.tensor.matmul(pz[0:C, :], w3_sb[:], st["y2"][:],
                                 start=True, stop=True)
                ex = ep.tile([C, NB], F16, tag="ex")
                nc.scalar.activation(ex[:], pz[0:C, :], AF.Exp,
                                     bias=b3_sb[:, 0:1])
                st["pz"], st["ex"] = pz, ex
                return st

            def emit_tail(st):
                pz, ex, bs = st["pz"], st["ex"], st["bs"]
                nc.tensor.matmul(pz[32:33, :], ones_sb[:], ex[:],
                                 start=True, stop=True)
                # bufs=16: store receipts lag ~2 chunks; with fewer
                # buffers the DVE eviction stalls on them.
                zs = ep.tile([33, NB], F16, tag="zs", bufs=16)
                nc.vector.tensor_scalar(zs[:], pz[:], 1.0, 0.0,
                                        op0=ALU.mult, op1=ALU.add)
                nc.sync.dma_start(zd_d[:, bs:bs + NB], zs[0:C, :])
                nc.sync.dma_start(sd_d[:, bs:bs + NB], zs[32:33, :])

            PAIRS = ((6,), (0, 1), (2, 3), (4, 5))
            BPOS = {6: 0, 0: 1, 1: 2, 2: 3, 3: 4, 4: 5, 5: 6}
            stage_l2 = stage_l3 = stage_lse = None
            for s in range(n_sup):
                xs_t, xb_t = xs_tiles[s], xb_tiles[s]
                bs = s * SC
                xs_v = xs_t[:].rearrange("p (two c) -> p two c", two=2)

                y1 = hp.tile([OT, n_blk, NB], F8, name="y1", tag="y1")
                for idx, pair in enumerate(PAIRS):
                    if len(pair) == 2:
                        p = psp.tile([OT, 2 * NB], F32, tag="pp", name="pp")
                    else:
                        p = ps6.tile([OT, NB], F32, tag="p6", name="p6")
                    for bi, j in enumerate(pair):
                        nc.tensor.matmul(
                            p[:, bi * NB:(bi + 1) * NB],
                            wsv[:, BPOS[j], :, :],
                            xs_v[:], start=True, stop=False,
                            perf_mode=PM)
                    for bi, j in enumerate(pair):
                        kj = R_lens[j] + 1        # band cols + ones row
                        bj = BPOS[j]
                        nc.tensor.matmul(
                            p[:, bi * NB:(bi + 1) * NB],
                            wb_sb[:kj, bj * OT:(bj + 1) * OT],
                            xb_t[:kj, j * SC:j * SC + NB],
                            start=False, stop=True)
                    for bi, j in enumerate(pair):
                        # evictions: ACT takes blocks 0,2,4 (+exp);
                        # DVE takes 1,3,5,6 (+y2, z/S merge)
                        reg = p[:, bi * NB:(bi + 1) * NB]
                        if j % 2 == 0 and j != 6:
                            nc.scalar.activation(y1[:, j, :], reg, AF.Relu,
                                                 scale=1.0 / SW)
                        else:
                            nc.vector.tensor_scalar(y1[:, j, :], reg,
                                                    1.0 / SW, 0.0,
                                                    op0=ALU.mult,
                                                    op1=ALU.max)
                    if idx == 1:
                        if stage_lse is not None:
                            emit_tail(stage_lse)
                            stage_lse = None
                        if stage_l2 is not None:
                            stage_l3 = emit_l2(stage_l2)
                            stage_l2 = None
                    elif idx == 3:
                        if stage_l3 is not None:
                            stage_lse = emit_l3(stage_l3)
                            stage_l3 = None
                    if s == 0:
                        # chunk 0 is paced by the DMA ramp: fill the
                        # known feed-lag windows so the HAM clock gate
                        # never sees an idle window and re-throttles.
                        dummy_mms(3)

                stage_l2 = {"y1": y1, "bs": bs}
                if s == 0:
                    dummy_mms(16)
                elif s == 1:
                    dummy_mms(4)

            if stage_lse is not None:
                emit_tail(stage_lse)
            emit_tail(emit_l3(emit_l2(stage_l2)))

    nc.compile()
    return nc


_CACHE = {}


def _prepare(x, W1, b1, W2, b2, W3, b3, mask1, mask2, mask3):
    B, D1 = x.shape
    H = W2.shape[0]
    C = W3.shape[0]
    assert B % N_CORES == 0
    Bc = B // N_CORES

    S, R_list = _decompose_mask1(np.asarray(mask1))
    R_lens = [len(r) for r in R_list]
    n_blk = len(R_list)
    Pb = max(R_lens) + 1
    nS = len(S)
    assert nS % 2 == 0
    hw = nS // 2
    n_sup = Bc // SC

    Wm1 = (np.asarray(W1) * np.asarray(mask1)).astype(np.float32)
    Wm2 = (np.asarray(W2) * np.asarray(mask2)).astype(np.float32)
    Wm3 = (np.asarray(W3) * np.asarray(mask3)).astype(np.float32)
    b1 = np.asarray(b1, np.float32)

    # stripe weights, BLOCK-MAJOR [hw, n_blk, 2, OT] fp8, x16, with the
    # kernel's pair order (6,0,1,..,5) so chunk 0 streams in slivers
    blk_order = [6, 0, 1, 2, 3, 4, 5]
    ws = np.zeros((hw, 2, D1), np.float32)
    ws[:, 0, :] = Wm1[:, S[:hw]].T * SW
    ws[:, 1, :] = Wm1[:, S[hw:]].T * SW
    ws_bm = np.zeros((hw, n_blk, 2, OT), np.float32)
    for i, j in enumerate(blk_order):
        ws_bm[:, i] = ws[:, :, j * OT:(j + 1) * OT]
    ws8 = ws_bm.astype(E4NP).reshape(hw, 2 * D1)

    # band weights, same block order [Pb, n_blk*OT] fp8, x16, with b1*16
    # in the ones-row
    wb = np.zeros((Pb, n_blk * OT), np.float32)
    for i, j in enumerate(blk_order):
        R = R_list[j]
        wb[:len(R), i * OT:(i + 1) * OT] = Wm1[j * OT:(j + 1) * OT, R].T * SW
        wb[len(R), i * OT:(i + 1) * OT] = b1[j * OT:(j + 1) * OT] * SW
    wb8 = wb.astype(E4NP)

    # L2 weights [OT, n_blk, H] fp8, x16: DR pairs (0,1)(2,3)(4,5) +
    # plain block 6
    n_kc2 = D1 // OT
    assert n_kc2 == n_blk
    w2t = Wm2.T.reshape(n_kc2, OT, H)     # [7, 112, H]
    w2 = np.zeros((OT, n_blk, H), np.float32)
    for k in range(n_blk):
        w2[:, k, :] = w2t[k] * SW
    w28 = w2.astype(E4NP).reshape(OT, n_blk * H)

    # y2 is stored at x16 scale (the DVE eviction has no spare op for the
    # 1/16), so w3 absorbs the 1/16 and b2 arrives pre-multiplied by 16.
    w316 = np.ascontiguousarray(Wm3.T / SW).astype(np.float16)   # [H, C]
    b2p = (np.asarray(b2, np.float32) * SW).reshape(H, 1)
    b3p = np.asarray(b3, np.float32).reshape(C, 1)

    xT = np.asarray(x, np.float32).T                        # [D1, B]
    # stripe x [hw, 2, B] fp8 -> per-core chunk slabs
    xs_all = np.stack([xT[S[:hw]], xT[S[hw:]]], axis=1).astype(E4NP)
    xs_all = np.ascontiguousarray(
        xs_all.reshape(hw, 2, N_CORES, n_sup, SC).transpose(0, 2, 3, 1, 4))
    # band x [Pb, n_blk, B] fp8 with ones-row at index len(R_j)
    xb_all = np.zeros((Pb, n_blk, B), E4NP)
    for j, R in enumerate(R_list):
        xb_all[:len(R), j] = xT[R].astype(E4NP)
        xb_all[len(R), j] = 1.0
    xb_all = np.ascontiguousarray(
        xb_all.reshape(Pb, n_blk, N_CORES, n_sup, SC).transpose(0, 2, 3, 1, 4))

    meta = dict(nS=nS, R_lens=R_lens, Pb=Pb, Bc=Bc, D1=D1, H=H, C=C,
                b3=np.asarray(b3, np.float32).reshape(C))
    key = (B, D1, H, C, nS, tuple(R_lens))
    if key not in _CACHE:
        _CACHE[key] = _build_program(meta)
    nc = _CACHE[key]

    in_maps = []
    for c in range(N_CORES):
        in_maps.append({
            "xs": xs_all[:, c].reshape(hw, n_sup * 2 * SC),
            "xb": xb_all[:, c].reshape(Pb, n_sup * n_blk * SC),
            "ws": ws8, "wb": wb8, "w2": w28, "w3": w316,
            "b2": b2p, "b3": b3p,
            "ones": np.ones((C, 1), np.float16),
        })
    return nc, in_maps, meta


def _assemble(results, meta):
    zs = [np.asarray(results[c]["zd"], np.float32).T      # [Bc, C]
          for c in range(N_CORES)]
    ss = [np.asarray(results[c]["sd"], np.float32).reshape(-1)
          for c in range(N_CORES)]
    z = np.concatenate(zs, axis=0)
    S = np.concatenate(ss, axis=0)
    out = z + meta["b3"][None, :] - np.log(S)[:, None]
    return out.astype(np.float32)


def kernel(**inputs):
    nc, in_maps, meta = _prepare(**inputs)
    res = bass_utils.run_bass_kernel_spmd(nc, in_maps,
                                          core_ids=list(range(N_CORES)))
    return _assemble(res.results, meta)


def kernel_traced(tmpdir=None, **inputs):
    """Same as kernel() but with NTFF profiling; returns (output, results)."""
    nc, in_maps, meta = _prepare(**inputs)
    res = bass_utils.run_bass_kernel_spmd(nc, in_maps,
                                          core_ids=list(range(N_CORES)),
                                          trace=True, tmpdir=tmpdir)
    return _assemble(res.results, meta), res


# revision 42
# speedup vs baseline: 1.0604x; 1.0193x over previous
"""Trainium2 Bass kernel for the ButterflyMlp problem.

Computes log_softmax(L3(relu(L2(relu(L1(x)))))) where each Li is a masked
linear layer (butterfly sparsity: global column stripes + a diagonal band),
batch 65536, data-parallel over 8 NeuronCores (8192 rows/core).

Strategy (per core, feature-major throughout):
  - Masks are pre-applied to weights on host. Layer-1 splits into the dense
    stripe GEMM (204 columns shared by all outputs) and a narrow per-block
    band GEMM (<=93 residual columns per 112-row output block).
  - Pass cost on the PE is ~N cycles (N=512 moving columns) regardless of K,
    so the kernel minimizes pass count: fp8e4 DoubleRow contracts 2x128 rows
    per pass. Per 512-column chunk: 7 stripe DR + 7 band plain (L1),
    3 DR + 1 plain (L2: pairs (0,1)(2,3)(4,5) + block 6), 1 fp16 pass (L3),
    1 fp16 pass (exp-sum). Weights are scaled x16 before fp8 quantization;
    the 1/16 folds into eviction scales / w3.
  - HAM clock gate: the PE runs at 1.2 GHz until ~3.4us of *continuous*
    busy time, and any >3.4us idle gap re-throttles it. So the DMA stream
    is ordered to start the first stripe matmul as early as possible
    (per-block ws slivers, then chunk-0 x) and all 16 chunks of x are
    prefetched with 16-deep tile buffers so the PE never starves mid-run.
  - Per chunk, stripe matmuls are emitted in two groups of 4/3 ahead of
    their band matmuls (PSUM limit: 3 pair tiles + 1 single + L2 + L3
    banks = 8) so chunk 0 can start on ws+xs alone before wb/xb land.
  - log_softmax is finished on HOST: the kernel ships z = L3 out (fp16)
    and S = sum(exp(z+b3)) (fp16, via an M=1 ones-matmul into partition 32
    of the same PSUM bank as z, so one [33,512] eviction covers both);
    host computes z + b3 - ln(S). This drops the on-device ln/subtract.
  - Evictions (PSUM fp32 reads are capped at 1 elem/cycle/lane) are
    balanced 5 ops/chunk on ACT (4 relu blocks + exp) and 5 on DVE
    (3 relu blocks + y2 + z/S merged), each ~3.5us/chunk.
  - Bulk DMA via SWDGE (gpsimd ring) in strictly-2D patterns (3D falls
    back to slow GpSimd-ucode copies); small weights + z/S stores ride
    the HWDGE (sync) queue in parallel.
"""
import sys
sys.path.insert(0, "/opt/trn_rl_repo")
import numpy as np
import ml_dtypes

import concourse.bass as bass
import concourse.bacc as bacc
import concourse.mybir as mybir
import concourse.tile as tile
from concourse import bass_utils

F32 = mybir.dt.float32
F16 = mybir.dt.float16
F8 = mybir.dt.float8e4
E4NP = ml_dtypes.float8_e4m3
PM = mybir.MatmulPerfMode.DoubleRow
AF = mybir.ActivationFunctionType
ALU = mybir.AluOpType

# Keep Exp/Relu/Identity/Copy in one ACT table set so the greedy chooser
# emits a single table load instead of reloading twice per chunk.
_PIN_SET = "natural_log_exp_and_others"
_orig_gat = bacc.get_activation_tables


def _pinned_gat(arch):
    tabs = _orig_gat(arch)
    need = {AF.Relu, AF.Identity, AF.Exp, AF.Copy}
    if _PIN_SET in tabs and need <= tabs[_PIN_SET]:
        for name in tabs:
            if name != _PIN_SET:
                tabs[name] = tabs[name] - need
    return tabs


bacc.get_activation_tables = _pinned_gat

N_CORES = 8
NB = 512          # batch columns per matmul (one PSUM bank of fp32)
SC = 512          # batch columns per DMA chunk (= one matmul chunk)
OT = 112          # layer-1 output block width (784/7)
SW = 16.0         # weight pre-scale before fp8 quantization


def _chunk_plan(n_sup):
    """(offset, width) work items per core. Chunks 0-1 and the last chunk
    are processed as 256-column halves: at the start the DMA ramp stays
    ahead of the PE at finer granularity (no HAM re-throttle), and at the
    end the serial L2->L3->exp->lse->store drain is halved."""
    nbs = []
    for s in range(n_sup):
        if s <= 1 or s == n_sup - 1:
            nbs += [SC // 2, SC // 2]
        else:
            nbs.append(SC)
    plan, off = [], 0
    for nb in nbs:
        plan.append((off, nb))
        off += nb
    return plan


def _decompose_mask1(mask1):
    """Stripe columns S (true for every row) and per-block residuals R_j."""
    D_out, D_in = mask1.shape
    S = np.where(mask1.all(axis=0))[0]
    n_blk = (D_out + OT - 1) // OT
    stripe_set = np.zeros(D_in, dtype=bool)
    stripe_set[S] = True
    R_list = []
    for j in range(n_blk):
        blk = mask1[j * OT:(j + 1) * OT]
        cols = np.where(blk.any(axis=0) & ~stripe_set)[0]
        assert len(cols) <= 127, f"band block {j} has {len(cols)} cols"
        R_list.append(cols)
    return S, R_list


def _build_program(meta):
    nS, R_lens = meta["nS"], meta["R_lens"]
    Pb = meta["Pb"]                       # band partitions (max R_len + 1)
    Bc = meta["Bc"]
    D1, H, C = meta["D1"], meta["H"], meta["C"]
    n_blk = len(R_lens)
    assert nS % 2 == 0
    hw = nS // 2                          # stripe half width (102)
    n_sup = Bc // SC

    nc = bacc.Bacc("TRN2", target_bir_lowering=False, debug=False,
                   enable_asserts=False, num_devices=N_CORES)
    n_items = len(_chunk_plan(n_sup))

    xs_d = nc.dram_tensor("xs", [hw, n_sup * 2 * SC], F8,
                          kind="ExternalInput").ap()
    xb_d = nc.dram_tensor("xb", [Pb, n_sup * n_blk * SC], F8,
                          kind="ExternalInput").ap()
    ws_d = nc.dram_tensor("ws", [hw, 2 * D1], F8, kind="ExternalInput").ap()
    wb_d = nc.dram_tensor("wb", [Pb, n_blk * OT], F8,
                          kind="ExternalInput").ap()
    w2_d = nc.dram_tensor("w2", [OT, n_blk * H], F8,
                          kind="ExternalInput").ap()
    w3_d = nc.dram_tensor("w3", [H, C], F16, kind="ExternalInput").ap()
    b2_d = nc.dram_tensor("b2", [H, 1], F32, kind="ExternalInput").ap()
    b3_d = nc.dram_tensor("b3", [C, 1], F32, kind="ExternalInput").ap()
    ones_d = nc.dram_tensor("ones", [C, 1], F16, kind="ExternalInput").ap()
    zd_d = nc.dram_tensor("zd", [C, Bc], F16, kind="ExternalOutput").ap()
    sd_d = nc.dram_tensor("sd", [1, Bc], F16, kind="ExternalOutput").ap()

    with tile.TileContext(nc) as tc:
        with tc.tile_pool(name="wp", bufs=1) as wp, \
             tc.tile_pool(name="xp", bufs=n_items) as xp, \
             tc.tile_pool(name="hp", bufs=2) as hp, \
             tc.tile_pool(name="ep", bufs=2) as ep, \
             tc.tile_pool(name="psp", bufs=2, space="PSUM") as psp, \
             tc.tile_pool(name="ps6", bufs=1, space="PSUM") as ps6, \
             tc.tile_pool(name="ps2", bufs=1, space="PSUM") as ps2, \
             tc.tile_pool(name="psd", bufs=1, space="PSUM") as psd, \
             tc.tile_pool(name="psz", bufs=1, space="PSUM") as psz:

            # ---- HAM warm-up: ~14 dummy matmuls on an uninitialized
            # SBUF tile keep the PE continuously busy from the end of the
            # preamble, so the clock gate reaches 8/8 (~2.4 GHz) before
            # the first real chunk's x lands (~12us). Results land in a
            # dedicated PSUM bank and are never read.
            wdummy = wp.tile([128, NB], F8)
            nc.vector.memset(wdummy[:], 0)
            pd = psd.tile([128, NB], F32, tag="pd", name="pd")

            def dummy_mms(n):
                for _ in range(n):
                    nc.tensor.matmul(pd[:], wdummy[:, 0:128], wdummy[:],
                                     start=True, stop=True)

            dummy_mms(10)

            # ---- chunk-0-critical slivers ride the HWDGE (sync) queue —
            # no Q7 descriptor-generation latency, and they stream
            # concurrently with the SWDGE bulk stream below. ws/wb arrive
            # in HOST-REORDERED block-major layout (block order 6,0,..,5
            # = the kernel's pair order) so pair 6 needs only 184KB.
            plan = _chunk_plan(n_sup)
            ws_sb = wp.tile([hw, 2 * D1], F8)
            wsv = ws_sb[:].rearrange("p (blk two m) -> p blk two m",
                                     blk=n_blk, two=2)
            wb_sb = wp.tile([Pb, n_blk * OT], F8)
            xs_tiles, xb_tiles = [], []
            o0, nb0 = plan[0]
            xs_t0 = xp.tile([hw, 2 * nb0], F8, name="xs_t", tag="xs")
            xb_t0 = xp.tile([Pb, n_blk * nb0], F8, name="xb_t", tag="xb")
            BO = 2 * OT    # flat ws cols per block
            nc.sync.dma_start(ws_sb[:, 0:BO], ws_d[:, 0:BO])         # b6
            nc.sync.dma_start(xs_t0[:], xs_d[:, 0:2 * nb0])
            nc.sync.dma_start(wb_sb[:, 0:OT], wb_d[:, 0:OT])         # b6
            nc.sync.dma_start(xb_t0[:, 6 * nb0:7 * nb0],
                              xb_d[:, 6 * nb0:7 * nb0])              # b6
            w2_sb = wp.tile([OT, n_blk * H], F8)
            nc.sync.dma_start(w2_sb[:], w2_d[:])
            w3_sb = wp.tile([H, C], F16)
            nc.sync.dma_start(w3_sb[:], w3_d[:])
            b2_sb = wp.tile([H, 1], F32)
            nc.sync.dma_start(b2_sb[:], b2_d[:])
            b3_sb = wp.tile([C, 1], F32)
            nc.sync.dma_start(b3_sb[:], b3_d[:])
            ones_sb = wp.tile([C, 1], F16)
            nc.sync.dma_start(ones_sb[:], ones_d[:])
            w2_v = w2_sb[:].rearrange("p (blk h) -> p blk h", blk=n_blk)

            # ---- SWDGE bulk stream (strictly-2D patterns only); all x
            # tiles stay resident.
            nc.gpsimd.dma_start(ws_sb[:, BO:n_blk * BO],
                                ws_d[:, BO:n_blk * BO])
            nc.gpsimd.dma_start(xb_t0[:, 0:2 * nb0], xb_d[:, 0:2 * nb0])
            nc.gpsimd.dma_start(wb_sb[:, OT:n_blk * OT],
                                wb_d[:, OT:n_blk * OT])
            nc.gpsimd.dma_start(xb_t0[:, 2 * nb0:6 * nb0],
                                xb_d[:, 2 * nb0:6 * nb0])
            xs_tiles.append(xs_t0)
            xb_tiles.append(xb_t0)
            for off, nb in plan[1:]:
                xs_t = xp.tile([hw, 2 * nb], F8, name="xs_t", tag="xs")
                nc.gpsimd.dma_start(
                    xs_t[:], xs_d[:, 2 * off:2 * (off + nb)])
                xb_t = xp.tile([Pb, n_blk * nb], F8, name="xb_t", tag="xb")
                nc.gpsimd.dma_start(
                    xb_t[:], xb_d[:, n_blk * off:n_blk * (off + nb)])
                xs_tiles.append(xs_t)
                xb_tiles.append(xb_t)

            # The L2->L3->softmax tail is software-pipelined across chunks
            # so the PE never waits on an eviction: during chunk s's L1
            # phase the kernel emits lse(s-2)+stores and L2(s-1) after
            # pair01, and L3(s-1)+exp after pair45. Pair 6 runs FIRST so
            # its eviction lands early (it feeds L2's plain pass). Every
            # cross-engine edge gets >=0.7us of slack. z and S share one
            # PSUM bank (z at partitions 0..9, S at 32), so a single
            # [33,NB] DVE eviction covers both; host does z + b3 - ln(S).
            def emit_l2(st):
                y1, nb = st["y1"], st["nb"]
                p2 = ps2.tile([H, nb], F32, tag="l2", name="p2")
                for kp in range(3):
                    nc.tensor.matmul(p2[:],
                                     w2_v[:, 2 * kp:2 * kp + 2, :],
                                     y1[:, 2 * kp:2 * kp + 2, :],
                                     start=(kp == 0), stop=False,
                                     perf_mode=PM)
                nc.tensor.matmul(p2[:], w2_v[:, 6, :], y1[:, 6, :],
                                 start=False, stop=True)
                # y2 stored at x16 scale (w3 pre-divided by 16 on host);
                # b2 arrives pre-multiplied by 16.
                y2 = hp.tile([H, nb], F16, tag="y2")
                nc.vector.tensor_scalar(y2[:], p2[:], b2_sb[:, 0:1], 0.0,
                                        op0=ALU.add, op1=ALU.max)
                st["y2"] = y2
                return st

            def emit_l3(st):
                nb = st["nb"]
                pz = psz.tile([33, nb], F32, tag="l3", name="pz")
                nc.tensor.matmul(pz[0:C, :], w3_sb[:], st["y2"][:],
                                 start=True, stop=True)
                ex = ep.tile([C, nb], F16, tag="ex")
                nc.scalar.activation(ex[:], pz[0:C, :], AF.Exp,
                                     bias=b3_sb[:, 0:1])
                st["pz"], st["ex"] = pz, ex
                return st

            def emit_tail(st):
                pz, ex, bs, nb = st["pz"], st["ex"], st["bs"], st["nb"]
                nc.tensor.matmul(pz[32:33, :], ones_sb[:], ex[:],
                                 start=True, stop=True)
                # high bufs: store receipts lag ~2 chunks; with fewer
                # buffers the DVE eviction stalls on them.
                zs = ep.tile([33, nb], F16, tag="zs", bufs=16)
                nc.vector.tensor_scalar(zs[:], pz[:], 1.0, 0.0,
                                        op0=ALU.mult, op1=ALU.add)
                nc.sync.dma_start(zd_d[:, bs:bs + nb], zs[0:C, :])
                nc.sync.dma_start(sd_d[:, bs:bs + nb], zs[32:33, :])

            PAIRS = ((6,), (0, 1), (2, 3), (4, 5))
            BPOS = {6: 0, 0: 1, 1: 2, 2: 3, 3: 4, 4: 5, 5: 6}
            stage_l2 = stage_l3 = stage_lse = None
            for it, (bs, nb) in enumerate(plan):
                xs_t, xb_t = xs_tiles[it], xb_tiles[it]
                xs_v = xs_t[:].rearrange("p (two c) -> p two c", two=2)

                y1 = hp.tile([OT, n_blk, nb], F8, name="y1", tag="y1")
                for idx, pair in enumerate(PAIRS):
                    # pair tiles always span 2 PSUM banks with each block
                    # at its own bank: start=True zeroes PSUM at 2KB
                    # granularity, so two accumulation groups must never
                    # share a bank row.
                    if len(pair) == 2:
                        p = psp.tile([OT, 2 * NB], F32, tag="pp", name="pp")
                    else:
                        p = ps6.tile([OT, nb], F32, tag="p6", name="p6")
                    for bi, j in enumerate(pair):
                        nc.tensor.matmul(
                            p[:, bi * NB:bi * NB + nb],
                            wsv[:, BPOS[j], :, :],
                            xs_v[:], start=True, stop=False,
                            perf_mode=PM)
                    for bi, j in enumerate(pair):
                        kj = R_lens[j] + 1        # band cols + ones row
                        bj = BPOS[j]
                        nc.tensor.matmul(
                            p[:, bi * NB:bi * NB + nb],
                            wb_sb[:kj, bj * OT:(bj + 1) * OT],
                            xb_t[:kj, j * nb:(j + 1) * nb],
                            start=False, stop=True)
                    for bi, j in enumerate(pair):
                        # evictions: ACT takes blocks 0,2,4 (+exp);
                        # DVE takes 1,3,5,6 (+y2, z/S merge)
                        reg = p[:, bi * NB:bi * NB + nb]
                        if j % 2 == 0 and j != 6:
                            nc.scalar.activation(y1[:, j, :], reg, AF.Relu,
                                                 scale=1.0 / SW)
                        else:
                            nc.vector.tensor_scalar(y1[:, j, :], reg,
                                                    1.0 / SW, 0.0,
                                                    op0=ALU.mult,
                                                    op1=ALU.max)
                    if idx == 1:
                        if stage_lse is not None:
                            emit_tail(stage_lse)
                            stage_lse = None
                        if stage_l2 is not None:
                            stage_l3 = emit_l2(stage_l2)
                            stage_l2 = None
                    elif idx == 3:
                        if stage_l3 is not None:
                            stage_lse = emit_l3(stage_l3)
                            stage_l3 = None
                    if it == 0:
                        # item 0 is paced by the DMA ramp: fill the
                        # known feed-lag windows so the HAM clock gate
                        # never sees an idle window and re-throttles.
                        dummy_mms(2)

                stage_l2 = {"y1": y1, "bs": bs, "nb": nb}
                if it == 0:
                    dummy_mms(6)
                elif it == 1:
                    dummy_mms(4)
                elif it in (2, 3):
                    dummy_mms(2)

            if stage_lse is not None:
                emit_tail(stage_lse)
            emit_tail(emit_l3(emit_l2(stage_l2)))

    nc.compile()
    return nc


_CACHE = {}


def _prepare(x, W1, b1, W2, b2, W3, b3, mask1, mask2, mask3):
    B, D1 = x.shape
    H = W2.shape[0]
    C = W3.shape[0]
    assert B % N_CORES == 0
    Bc = B // N_CORES

    S, R_list = _decompose_mask1(np.asarray(mask1))
    R_lens = [len(r) for r in R_list]
    n_blk = len(R_list)
    Pb = max(R_lens) + 1
    nS = len(S)
    assert nS % 2 == 0
    hw = nS // 2
    n_sup = Bc // SC

    Wm1 = (np.asarray(W1) * np.asarray(mask1)).astype(np.float32)
    Wm2 = (np.asarray(W2) * np.asarray(mask2)).astype(np.float32)
    Wm3 = (np.asarray(W3) * np.asarray(mask3)).astype(np.float32)
    b1 = np.asarray(b1, np.float32)

    # stripe weights, BLOCK-MAJOR [hw, n_blk, 2, OT] fp8, x16, with the
    # kernel's pair order (6,0,1,..,5) so chunk 0 streams in slivers
    blk_order = [6, 0, 1, 2, 3, 4, 5]
    ws = np.zeros((hw, 2, D1), np.float32)
    ws[:, 0, :] = Wm1[:, S[:hw]].T * SW
    ws[:, 1, :] = Wm1[:, S[hw:]].T * SW
    ws_bm = np.zeros((hw, n_blk, 2, OT), np.float32)
    for i, j in enumerate(blk_order):
        ws_bm[:, i] = ws[:, :, j * OT:(j + 1) * OT]
    ws8 = ws_bm.astype(E4NP).reshape(hw, 2 * D1)

    # band weights, same block order [Pb, n_blk*OT] fp8, x16, with b1*16
    # in the ones-row
    wb = np.zeros((Pb, n_blk * OT), np.float32)
    for i, j in enumerate(blk_order):
        R = R_list[j]
        wb[:len(R), i * OT:(i + 1) * OT] = Wm1[j * OT:(j + 1) * OT, R].T * SW
        wb[len(R), i * OT:(i + 1) * OT] = b1[j * OT:(j + 1) * OT] * SW
    wb8 = wb.astype(E4NP)

    # L2 weights [OT, n_blk, H] fp8, x16: DR pairs (0,1)(2,3)(4,5) +
    # plain block 6
    n_kc2 = D1 // OT
    assert n_kc2 == n_blk
    w2t = Wm2.T.reshape(n_kc2, OT, H)     # [7, 112, H]
    w2 = np.zeros((OT, n_blk, H), np.float32)
    for k in range(n_blk):
        w2[:, k, :] = w2t[k] * SW
    w28 = w2.astype(E4NP).reshape(OT, n_blk * H)

    # y2 is stored at x16 scale (the DVE eviction has no spare op for the
    # 1/16), so w3 absorbs the 1/16 and b2 arrives pre-multiplied by 16.
    w316 = np.ascontiguousarray(Wm3.T / SW).astype(np.float16)   # [H, C]
    b2p = (np.asarray(b2, np.float32) * SW).reshape(H, 1)
    b3p = np.asarray(b3, np.float32).reshape(C, 1)

    xT = np.asarray(x, np.float32).T                        # [D1, B]
    # stripe x [hw, 2, B] fp8; band x [Pb, n_blk, B] fp8 with the
    # ones-row at index len(R_j). Both are packed per core into
    # plan-ordered slabs ([hw, 2, nb] / [Pb, n_blk, nb] flattened).
    plan = _chunk_plan(n_sup)
    xs_full = np.stack([xT[S[:hw]], xT[S[hw:]]], axis=1).astype(E4NP)
    xb_full = np.zeros((Pb, n_blk, B), E4NP)
    for j, R in enumerate(R_list):
        xb_full[:len(R), j] = xT[R].astype(E4NP)
        xb_full[len(R), j] = 1.0
    xs_cores, xb_cores = [], []
    for c in range(N_CORES):
        xc = xs_full[:, :, c * Bc:(c + 1) * Bc]
        bc = xb_full[:, :, c * Bc:(c + 1) * Bc]
        xs_cores.append(np.concatenate(
            [xc[:, :, off:off + nb].reshape(hw, 2 * nb)
             for off, nb in plan], axis=1))
        xb_cores.append(np.concatenate(
            [bc[:, :, off:off + nb].reshape(Pb, n_blk * nb)
             for off, nb in plan], axis=1))

    meta = dict(nS=nS, R_lens=R_lens, Pb=Pb, Bc=Bc, D1=D1, H=H, C=C,
                b3=np.asarray(b3, np.float32).reshape(C))
    key = (B, D1, H, C, nS, tuple(R_lens))
    if key not in _CACHE:
        _CACHE[key] = _build_program(meta)
    nc = _CACHE[key]

    in_maps = []
    for c in range(N_CORES):
        in_maps.append({
            "xs": np.ascontiguousarray(xs_cores[c]),
            "xb": np.ascontiguousarray(xb_cores[c]),
            "ws": ws8, "wb": wb8, "w2": w28, "w3": w316,
            "b2": b2p, "b3": b3p,
            "ones": np.ones((C, 1), np.float16),
        })
    return nc, in_maps, meta


def _assemble(results, meta):
    zs = [np.asarray(results[c]["zd"], np.float32).T      # [Bc, C]
          for c in range(N_CORES)]
    ss = [np.asarray(results[c]["sd"], np.float32).reshape(-1)
          for c in range(N_CORES)]
    z = np.concatenate(zs, axis=0)
    S = np.concatenate(ss, axis=0)
    out = z + meta["b3"][None, :] - np.log(S)[:, None]
    return out.astype(np.float32)


def kernel(**inputs):
    nc, in_maps, meta = _prepare(**inputs)
    res = bass_utils.run_bass_kernel_spmd(nc, in_maps,
                                          core_ids=list(range(N_CORES)))
    return _assemble(res.results, meta)


def kernel_traced(tmpdir=None, **inputs):
    """Same as kernel() but with NTFF profiling; returns (output, results)."""
    nc, in_maps, meta = _prepare(**inputs)
    res = bass_utils.run_bass_kernel_spmd(nc, in_maps,
                                          core_ids=list(range(N_CORES)),
                                          trace=True, tmpdir=tmpdir)
    return _assemble(res.results, meta), res
